# revision 32
# baseline (speedup 1.0000x reference)
"""Trainium2 Bass kernel for an enhanced transformer block (attn + depthwise-conv + MLP).

Sharding: 8 cores = 4 batches x 2 sequence halves (data parallel, no collectives).
Each core receives its batch's x TRANSPOSED (feature-major: d on partitions,
tokens on the free axis) and ROTATED so that its extended token range
[t0-1, t1+1) lands at columns [0, 1026) uniformly on every core (SPMD: one
program, different data). K/V are computed over the full (rotated) sequence;
q/attention only over the core's 1026 extended columns. The rotation makes
attention sums run over a permuted key order, which is mathematically
identical. Halo columns provide the depthwise-conv neighbor values; at
sequence edges the halo is dead (wrapped garbage) and is zeroed via a mask
folded into LN2's rstd.

Softmax is computed without max-subtraction (scores for this problem are
O(1); exp cannot overflow) so the denominator can be accumulated by an
extra all-ones column appended to V in the P@V matmul.
"""

import numpy as np
import ml_dtypes

import concourse.bass as bass
import concourse.bacc as bacc
import concourse.mybir as mybir
import concourse.tile as tile
from concourse.bass_utils import run_bass_kernel_spmd

F32 = mybir.dt.float32
F32R = mybir.dt.float32r
BF16 = mybir.dt.bfloat16
Alu = mybir.AluOpType
Act = mybir.ActivationFunctionType

D = 512          # model dim
S = 2048         # sequence length
B = 4            # batch
H = 8            # heads
HD = 64          # head dim
DFF = 2048       # mlp hidden
NCORES = 8
TLOC = 1024      # local tokens per core
TEXT = 1026      # extended (1 halo col each side)
DT = 4           # d-tiles of 128
EPS = 1e-5

# order of packed 512-length vectors in the "vecs" input
VEC_NAMES = ["ln1_g", "ln1_b", "ln2_g", "ln2_b", "lnc_g", "lnc_b",
             "ln3_g", "ln3_b", "cw0", "cw1", "cw2", "cb",
             "bo_eff", "bq", "bk", "b2"]
VIDX = {n: i for i, n in enumerate(VEC_NAMES)}


def _vap(vecs_sb, name, dt):
    """per-partition [128,1] scalar AP for vector `name`, d-tile dt."""
    c = 4 * VIDX[name] + dt
    return vecs_sb[:, c:c + 1]


def build_program(flags, stage=6):
    """Trace the uniform per-core program. flags: dict of bools enabling
    optional bias/scale terms (specialized to the actual input values).
    stage<6 emits an intermediate tensor and stops (debug bisection)."""
    nc = bacc.Bacc("TRN2", target_bir_lowering=False, debug=False)

    xT_d = nc.dram_tensor("xT", (DT, 128, S), F32, kind="ExternalInput").ap()
    wqkvT_d = nc.dram_tensor("wqkvT", (DT, 128, 3 * D), BF16, kind="ExternalInput").ap()
    woT_d = nc.dram_tensor("woT", (DT, 128, D), BF16, kind="ExternalInput").ap()
    w1T_d = nc.dram_tensor("w1T", (DT, 128, DFF), BF16, kind="ExternalInput").ap()
    w2T_d = nc.dram_tensor("w2T", (16, 128, D), BF16, kind="ExternalInput").ap()
    vecs_d = nc.dram_tensor("vecs", (128, 4 * len(VEC_NAMES)), F32, kind="ExternalInput").ap()
    b1m_d = nc.dram_tensor("b1m", (128, 16), F32, kind="ExternalInput").ap()
    mask_d = nc.dram_tensor("mask", (128, TEXT), BF16, kind="ExternalInput").ap()
    yT_d = nc.dram_tensor("yT", (DT, 128, TLOC), F32, kind="ExternalOutput").ap()

    with tile.TileContext(nc) as tc:
        _prog(nc, tc, flags,
              xT_d, wqkvT_d, woT_d, w1T_d, w2T_d, vecs_d, b1m_d, mask_d, yT_d,
              stage=stage)
    nc.compile()
    return nc


def _ln_stats(nc, lnps, lnw, ones, eps_sb, z_tiles, sl, n):
    """LN stats over the d axis (partitions x 4 tiles) for token cols `sl`
    (length n). Returns (mu_rep, r_rep) fp32 SBUF tiles (128, n), replicated
    across partitions. z_tiles: 4 fp32 SBUF tiles (128, >=n cols)."""
    s1 = lnps.tile((128, 512), F32, name="s1", tag="s1", bufs=2)
    s2 = lnps.tile((128, 512), F32, name="s2", tag="s2", bufs=2)
    for dt in range(DT):
        xb = lnw.tile((128, 512), BF16, name="xb", tag="xb", bufs=4)
        nc.vector.tensor_copy(xb[:, :n], z_tiles[dt][:, sl])
        nc.tensor.matmul(s1[:, :n], lhsT=ones, rhs=xb[:, :n],
                         start=(dt == 0), stop=(dt == DT - 1))
        sq = lnw.tile((128, 512), BF16, name="sq", tag="sq", bufs=4)
        with nc.allow_low_precision("bf16 x^2 for LN variance (error ~3e-4 rel)"):
            nc.vector.tensor_mul(sq[:, :n], z_tiles[dt][:, sl], z_tiles[dt][:, sl])
        nc.tensor.matmul(s2[:, :n], lhsT=ones, rhs=sq[:, :n],
                         start=(dt == 0), stop=(dt == DT - 1))
    mu = lnw.tile((128, 512), F32, name="mu", tag="mu")
    nc.vector.tensor_scalar_mul(out=mu[:, :n], in0=s1[:, :n], scalar1=1.0 / D)
    mu2 = lnw.tile((128, 512), F32, name="mu2", tag="scratch", bufs=3)
    nc.vector.tensor_mul(mu2[:, :n], mu[:, :n], mu[:, :n])
    m2s = lnw.tile((128, 512), F32, name="m2s", tag="scratch", bufs=3)
    nc.vector.tensor_scalar_mul(out=m2s[:, :n], in0=s2[:, :n], scalar1=1.0 / D)
    var = lnw.tile((128, 512), F32, name="var", tag="var")
    nc.vector.tensor_sub(var[:, :n], m2s[:, :n], mu2[:, :n])
    sd = lnw.tile((128, 512), F32, name="sd", tag="scratch", bufs=3)
    nc.scalar.activation(sd[:, :n], var[:, :n], Act.Sqrt, bias=eps_sb[:, 0:1])
    r = lnw.tile((128, 512), F32, name="r", tag="r")
    nc.vector.reciprocal(r[:, :n], sd[:, :n])
    return mu, r


def _ln_apply(nc, lnw, vecs_sb, z_tiles, out_tiles, sl, n, mu, r,
              gname, bname, gflag, bflag, out_sl=None):
    """out = (z - mu) * r [* g] [+ b] for each d-tile, cols sl."""
    osl = sl if out_sl is None else out_sl
    for dt in range(DT):
        xc = lnw.tile((128, 512), F32, name="xc", tag="xc", bufs=2)
        nc.vector.tensor_sub(xc[:, :n], z_tiles[dt][:, sl], mu[:, :n])
        dst = out_tiles[dt][:, osl]
        if gflag:
            nc.vector.scalar_tensor_tensor(out=dst, in0=xc[:, :n],
                                           scalar=_vap(vecs_sb, gname, dt),
                                           in1=r[:, :n], op0=Alu.mult, op1=Alu.mult)
        else:
            nc.vector.tensor_mul(dst, xc[:, :n], r[:, :n])
        if bflag:
            nc.vector.tensor_scalar_add(out=dst, in0=dst,
                                        scalar1=_vap(vecs_sb, bname, dt))


def _prog(nc, tc, fl, xT_d, wqkvT_d, woT_d, w1T_d, w2T_d, vecs_d, b1m_d,
          mask_d, yT_d, stage=6):
    Ls, Rs, Ps = [], [], []  # open-pool stacks (left / right / psum)

    def _dbg_exit(tiles):
        dbg = tc.alloc_tile_pool(name="dbgout", bufs=1)
        for dt in range(DT):
            t = dbg.tile((128, TLOC), F32, name=f"dbg{dt}", tag=f"dbg{dt}")
            nc.vector.tensor_copy(t, tiles[dt][:, 0:TLOC])
            nc.sync.dma_start(out=yT_d[dt], in_=t)
        dbg.release()
        for st in (Ps, Ls, Rs):
            while st:
                st.pop().release()

    # ---------------- persistent pools ----------------
    consts = tc.alloc_tile_pool(name="consts", bufs=1); Ls.append(consts)
    wts = tc.alloc_tile_pool(name="wts", bufs=1); Ls.append(wts)
    lnw = tc.alloc_tile_pool(name="lnw", bufs=2); Ls.append(lnw)
    small = tc.alloc_tile_pool(name="small", bufs=2); Ls.append(small)

    vecs_sb = consts.tile((128, 4 * len(VEC_NAMES)), F32, name="vecs_sb", tag="vecs")
    nc.sync.dma_start(out=vecs_sb, in_=vecs_d)
    b1_sb = consts.tile((128, 16), F32, name="b1_sb", tag="b1")
    nc.sync.dma_start(out=b1_sb, in_=b1m_d)
    mask_sb = consts.tile((128, TEXT), BF16, name="mask_sb", tag="mask")
    nc.sync.dma_start(out=mask_sb, in_=mask_d)
    ones = consts.tile((128, 128), BF16, name="ones", tag="ones")
    nc.vector.memset(ones, 1.0)
    eps_sb = consts.tile((128, 1), F32, name="eps_sb", tag="eps")
    nc.vector.memset(eps_sb, EPS)

    wqkv_sb = []
    for dt in range(DT):
        t = wts.tile((128, 3 * D), BF16, name=f"wqkv{dt}", tag=f"wqkv{dt}")
        nc.sync.dma_start(out=t, in_=wqkvT_d[dt])
        wqkv_sb.append(t)
    wo_sb = []
    for dt in range(DT):
        t = wts.tile((128, D), BF16, name=f"wo{dt}", tag=f"wo{dt}")
        nc.sync.dma_start(out=t, in_=woT_d[dt])
        wo_sb.append(t)

    # x_res: residual slice of x (cols 0:TEXT), outlives the full-x tiles
    xres_pool = tc.alloc_tile_pool(name="xres_pool", bufs=1, side="right"); Rs.append(xres_pool)
    xres_sb = [xres_pool.tile((128, TEXT), F32, name=f"xr{dt}", tag=f"xr{dt}")
               for dt in range(DT)]
    # aT (attention output, feature-major) - lives until out-proj
    a_pool = tc.alloc_tile_pool(name="a_pool", bufs=1, side="right"); Rs.append(a_pool)
    a_sb = [a_pool.tile((128, TEXT), BF16, name=f"a{dt}", tag=f"a{dt}")
            for dt in range(DT)]
    # k/v/q - live until end of attention
    kvq = tc.alloc_tile_pool(name="kvq", bufs=1, side="right"); Rs.append(kvq)

    # hT (LN1 output, bf16) - lives until end of QKV
    h_pool = tc.alloc_tile_pool(name="h_pool", bufs=1); Ls.append(h_pool)
    h_sb = [h_pool.tile((128, S), BF16, name=f"h{dt}", tag=f"h{dt}")
            for dt in range(DT)]

    # x tiles (feature-major, rotated), full sequence
    x_pool = tc.alloc_tile_pool(name="x_pool", bufs=1); Ls.append(x_pool)
    x_sb = []
    for dt in range(DT):
        t = x_pool.tile((128, S), F32, name=f"x{dt}", tag=f"x{dt}")
        nc.sync.dma_start(out=t, in_=xT_d[dt])
        x_sb.append(t)

    # ---------------- phase 1: LN1 over full sequence -> hT (bf16) --------
    ln1ps = tc.alloc_tile_pool(name="ln1ps", bufs=2, space="PSUM"); Ps.append(ln1ps)
    with nc.named_scope("ln1"):
        for ch in range(4):
            sl = slice(ch * 512, ch * 512 + 512)
            mu, r = _ln_stats(nc, ln1ps, lnw, ones, eps_sb, x_sb, sl, 512)
            _ln_apply(nc, lnw, vecs_sb, x_sb, h_sb, sl, 512, mu, r,
                      "ln1_g", "ln1_b", fl["ln1_g"], fl["ln1_b"])
    Ps.pop().release()
    for dt in range(DT):
        nc.vector.tensor_copy(xres_sb[dt], x_sb[dt][:, 0:TEXT])
    Ls.pop().release()  # x_pool
    if stage == 1:
        return _dbg_exit(h_sb)

    # ---------------- phase 2: QKV ----------------
    k_sb = [kvq.tile((128, S), BF16, name=f"k{dt}", tag=f"k{dt}") for dt in range(DT)]
    v_sb = [kvq.tile((128, H, HD + 1), BF16, name=f"v{tc_}", tag=f"v{tc_}")
            for tc_ in range(16)]
    q_sb = [kvq.tile((128, TEXT), BF16, name=f"q{dt}", tag=f"q{dt}")
            for dt in range(DT)]

    qkvps = tc.alloc_tile_pool(name="qkvps", bufs=4, space="PSUM"); Ps.append(qkvps)
    with nc.named_scope("qkv"):
        # k: feature-major (j on partitions, tokens free)
        for jt in range(DT):
            for ch in range(4):
                sl = slice(ch * 512, ch * 512 + 512)
                ps = qkvps.tile((128, 512), F32, name="kps", tag="mm")
                for dt in range(DT):
                    nc.tensor.matmul(ps, lhsT=wqkv_sb[dt][:, D + jt * 128: D + jt * 128 + 128],
                                     rhs=h_sb[dt][:, sl],
                                     start=(dt == 0), stop=(dt == DT - 1))
                if fl["bk"]:
                    nc.scalar.add(out=k_sb[jt][:, sl], in_=ps,
                                  add=_vap(vecs_sb, "bk", jt))
                else:
                    nc.scalar.copy(k_sb[jt][:, sl], ps)
        # q: feature-major, extended token range only
        for jt in range(DT):
            for (c0, n) in ((0, 512), (512, 512), (1024, 2)):
                tag = "mm" if n == 512 else "qtiny"
                ps = qkvps.tile((128, 512) if n == 512 else (128, 2), F32,
                                name="qps", tag=tag, bufs=4 if n == 512 else 2)
                for dt in range(DT):
                    nc.tensor.matmul(ps[:, :n], lhsT=wqkv_sb[dt][:, jt * 128: jt * 128 + 128],
                                     rhs=h_sb[dt][:, c0:c0 + n],
                                     start=(dt == 0), stop=(dt == DT - 1))
                if fl["bq"]:
                    nc.scalar.add(out=q_sb[jt][:, c0:c0 + n], in_=ps[:, :n],
                                  add=_vap(vecs_sb, "bq", jt))
                else:
                    nc.scalar.copy(q_sb[jt][:, c0:c0 + n], ps[:, :n])
        # v: token-major (tokens on partitions, j free), with ones column
        for tc_ in range(16):
            nc.vector.memset(v_sb[tc_][:, :, HD:HD + 1], 1.0)
            ps = qkvps.tile((128, 512), F32, name="vps", tag="mm")
            for dt in range(DT):
                nc.tensor.matmul(ps, lhsT=h_sb[dt][:, tc_ * 128: tc_ * 128 + 128],
                                 rhs=wqkv_sb[dt][:, 2 * D:3 * D],
                                 start=(dt == 0), stop=(dt == DT - 1))
            src = ps[:, :].rearrange("p (h d) -> p h d", h=H)
            # v bias would be per-free here; it is folded into bo_eff on host.
            nc.scalar.copy(v_sb[tc_][:, :, 0:HD], src)
    Ps.pop().release()  # qkvps
    Ls.pop().release()  # h_pool
    if stage == 2:
        return _dbg_exit(k_sb)

    # ---------------- phase 3: attention ----------------
    p_pool = tc.alloc_tile_pool(name="p_pool", bufs=4, side="right"); Rs.append(p_pool)
    scps = tc.alloc_tile_pool(name="scps", bufs=4, space="PSUM"); Ps.append(scps)
    avps = tc.alloc_tile_pool(name="avps", bufs=2, space="PSUM"); Ps.append(avps)

    with nc.named_scope("attn"):
        for hp in range(4):  # head pairs: a=2hp (rows 0:64), b=2hp+1 (rows 64:128)
            av_ab = [avps.tile((128, 1024), F32, name=f"av{hp}_{i}", tag="av")
                     for i in range(2)]
            rows = [slice(0, 64), slice(64, 128)]
            for kc in range(16):
                ksl = slice(kc * 128, kc * 128 + 128)
                ptiles = [None, None]
                for i in range(2):
                    sc = scps.tile((128, 1024), F32, name="sc", tag="sc", bufs=2)
                    for qc in range(2):
                        nc.tensor.matmul(sc[:, qc * 512:(qc + 1) * 512],
                                         lhsT=k_sb[hp][rows[i], ksl],
                                         rhs=q_sb[hp][rows[i], qc * 512:(qc + 1) * 512],
                                         start=True, stop=True)
                    pt = p_pool.tile((128, 1024), BF16, name="pt", tag="pt")
                    nc.scalar.activation(pt, sc, Act.Exp, scale=0.125)
                    ptiles[i] = pt
                # av accumulation
                for i in range(2):
                    for qc in range(2):
                        nc.tensor.matmul(av_ab[i][0:HD + 1, qc * 512:(qc + 1) * 512],
                                         lhsT=v_sb[kc][:, 2 * hp + i, :],
                                         rhs=ptiles[i][:, qc * 512:(qc + 1) * 512],
                                         start=(kc == 0), stop=(kc == 15))
            # normalize: recip of denominator row, replicate via K=1 matmul,
            # stage to SBUF (DVE reads only one PSUM operand), multiply
            for i in range(2):
                if stage == 31:
                    nc.vector.tensor_copy(a_sb[hp][rows[i], 0:1024],
                                          av_ab[i][0:64, :])
                    continue
                rec = small.tile((1, 1024), BF16, name="rec", tag="rec")
                with nc.allow_low_precision("bf16 softmax denom recip (attn out is tiny)"):
                    nc.vector.reciprocal(rec, av_ab[i][HD:HD + 1, :])
                for qc in range(2):
                    qsl = slice(qc * 512, qc * 512 + 512)
                    nc.tensor.matmul(av_ab[i][64:128, qsl],
                                     lhsT=ones[0:1, 0:64], rhs=rec[:, qsl],
                                     start=True, stop=True)
                rrep = small.tile((64, 1024), BF16, name="rrep", tag="rrep")
                nc.vector.tensor_copy(rrep, av_ab[i][64:128, :])
                nc.vector.tensor_tensor(a_sb[hp][rows[i], 0:1024],
                                        av_ab[i][0:64, :], rrep,
                                        Alu.mult)
    Ps.pop().release(); Ps.pop().release()  # avps scps
    Rs.pop().release()  # p_pool
    if stage in (3, 31, 32):
        Rs.pop().release()  # kvq
        return _dbg_exit(a_sb)

    # ---------------- phase 4: out-proj + residual -> x1 ----------------
    x2p = tc.alloc_tile_pool(name="x2p", bufs=1); Ls.append(x2p)
    x2_sb = [x2p.tile((128, TLOC), F32, name=f"x2_{dt}", tag=f"x2_{dt}")
             for dt in range(DT)]
    mid = tc.alloc_tile_pool(name="mid", bufs=1); Ls.append(mid)
    x1_sb = [mid.tile((128, TEXT), F32, name=f"x1_{dt}", tag=f"x1_{dt}")
             for dt in range(DT)]
    ops = tc.alloc_tile_pool(name="ops", bufs=3, space="PSUM"); Ps.append(ops)
    QC3 = ((0, 342), (342, 342), (684, 342))
    # -- halo attention (2 ext cols per core), token-major scores --
    phd_d = nc.dram_tensor("phd_scratch", (H, 2, S), BF16).ap()
    dsum_d = nc.dram_tensor("dsum_scratch", (H, 2, 1), F32).ap()
    hps = tc.alloc_tile_pool(name="hps", bufs=1, space="PSUM"); Ps.append(hps)
    hsb = tc.alloc_tile_pool(name="hsb", bufs=2)
    with nc.named_scope("halo"):
        for h in range(H):
            hp, i = h // 2, h % 2
            rws = slice(64 * i, 64 * i + 64)
            ph = hsb.tile((2, S), BF16, name="ph", tag="ph", bufs=1)
            dsum = hsb.tile((2, 2), F32, name="dsum", tag="dsum")
            for c2 in range(2):
                sch = hps.tile((2, 1024), F32, name="sch", tag="sch", bufs=1)
                for c in range(2):
                    cc = 2 * c2 + c
                    nc.tensor.matmul(sch[:, c * 512:(c + 1) * 512],
                                     lhsT=q_sb[hp][rws, 1024:1026],
                                     rhs=k_sb[hp][rws, cc * 512:(cc + 1) * 512],
                                     start=True, stop=True)
                nc.scalar.activation(ph[:, c2 * 1024:(c2 + 1) * 1024], sch,
                                     Act.Exp, scale=0.125,
                                     accum_out=dsum[:, c2:c2 + 1])
            nc.vector.tensor_add(dsum[:, 0:1], dsum[:, 0:1], dsum[:, 1:2])
            nc.sync.dma_start(out=phd_d[h], in_=ph)
            nc.sync.dma_start(out=dsum_d[h], in_=dsum[:, 0:1])
            pT = hsb.tile((128, 16, 2), BF16, name="pT", tag="pT")
            for q in range(2):
                nc.sync.dma_start(out=pT[:, :, q],
                                  in_=phd_d[h][q].rearrange("(c p) -> p c", p=128))
            denT = hsb.tile((1, 2), F32, name="denT", tag="denT")
            nc.sync.dma_start(out=denT, in_=dsum_d[h].rearrange("q one -> one q"))
            avh = hps.tile((128, 2), F32, name="avh", tag="avh", bufs=2)
            for kc in range(16):
                nc.tensor.matmul(avh[0:64, :], lhsT=v_sb[kc][:, h, 0:HD],
                                 rhs=pT[:, kc, :], start=(kc == 0), stop=(kc == 15))
            rec2 = hsb.tile((1, 2), BF16, name="rec2", tag="rec2")
            with nc.allow_low_precision("bf16 halo softmax recip"):
                nc.vector.reciprocal(rec2, denT)
            nc.tensor.matmul(avh[64:128, :], lhsT=ones[0:1, 0:64], rhs=rec2,
                             start=True, stop=True)
            rr2 = hsb.tile((64, 2), BF16, name="rr2", tag="rr2")
            nc.vector.tensor_copy(rr2, avh[64:128, :])
            nc.vector.tensor_tensor(a_sb[hp][rws, 1024:1026], avh[0:64, :],
                                    rr2, Alu.mult)
    hsb.release()
    Ps.pop().release()  # hps
    Rs.pop().release()  # kvq
    with nc.named_scope("outproj"):
        for jt in range(DT):
            for (c0, n) in QC3:
                sl = slice(c0, c0 + n)
                ps = ops.tile((128, 342), F32, name="ops_t", tag="o")
                for dt in range(DT):
                    nc.tensor.matmul(ps[:, :n], lhsT=wo_sb[dt][:, jt * 128: jt * 128 + 128],
                                     rhs=a_sb[dt][:, sl],
                                     start=(dt == 0), stop=(dt == DT - 1))
                if fl["bo"]:
                    nc.vector.scalar_tensor_tensor(out=x1_sb[jt][:, sl], in0=ps[:, :n],
                                                   scalar=_vap(vecs_sb, "bo_eff", jt),
                                                   in1=xres_sb[jt][:, sl],
                                                   op0=Alu.add, op1=Alu.add)
                else:
                    nc.vector.tensor_tensor(x1_sb[jt][:, sl], ps[:, :n],
                                            xres_sb[jt][:, sl], Alu.add)
    Ps.pop().release()  # ops
    Rs.pop().release()  # a_pool
    Rs.pop().release()  # xres_pool
    if stage == 4:
        return _dbg_exit(x1_sb)

    # ---------------- phase 5: conv block -> x2 ----------------
    h2_sb = [mid.tile((128, TEXT), F32, name=f"h2_{dt}", tag=f"h2_{dt}")
             for dt in range(DT)]
    conv_t = tc.alloc_tile_pool(name="conv_t", bufs=1); Ls.append(conv_t)
    tcv = [conv_t.tile((128, TLOC), F32, name=f"tc{dt}", tag=f"tc{dt}")
           for dt in range(DT)]
    g_sb = [conv_t.tile((128, TLOC), F32, name=f"g{dt}", tag=f"g{dt}")
            for dt in range(DT)]

    cps = tc.alloc_tile_pool(name="cps", bufs=2, space="PSUM"); Ps.append(cps)
    with nc.named_scope("convblock"):
        # LN2 over 1026 cols (3 chunks of 342), rstd masked at dead halo cols
        for (c0, n) in QC3:
            sl = slice(c0, c0 + n)
            mu, r = _ln_stats(nc, cps, lnw, ones, eps_sb, x1_sb, sl, n)
            nc.vector.tensor_mul(r[:, :n], r[:, :n], mask_sb[:, sl])
            _ln_apply(nc, lnw, vecs_sb, x1_sb, h2_sb, sl, n, mu, r,
                      "ln2_g", "ln2_b", fl["ln2_g"], fl["ln2_b"])
        # depthwise conv along tokens (output = local cols [1,1025) -> 1024)
        for dt in range(DT):
            tmp = conv_t.tile((128, TLOC), F32, name="ctmp", tag="ctmp", bufs=2)
            if fl["cb"]:
                nc.vector.tensor_scalar(out=tmp, in0=h2_sb[dt][:, 0:TLOC],
                                        scalar1=_vap(vecs_sb, "cw0", dt),
                                        scalar2=_vap(vecs_sb, "cb", dt),
                                        op0=Alu.mult, op1=Alu.add)
            else:
                nc.vector.tensor_scalar_mul(out=tmp, in0=h2_sb[dt][:, 0:TLOC],
                                            scalar1=_vap(vecs_sb, "cw0", dt))
            nc.vector.scalar_tensor_tensor(out=tmp, in0=h2_sb[dt][:, 1:TLOC + 1],
                                           scalar=_vap(vecs_sb, "cw1", dt),
                                           in1=tmp, op0=Alu.mult, op1=Alu.add)
            nc.vector.scalar_tensor_tensor(out=tcv[dt], in0=h2_sb[dt][:, 2:TLOC + 2],
                                           scalar=_vap(vecs_sb, "cw2", dt),
                                           in1=tmp, op0=Alu.mult, op1=Alu.add)
        # LNc on conv output (local 1024), then gelu
        for ch in range(2):
            sl = slice(ch * 512, ch * 512 + 512)
            mu, r = _ln_stats(nc, cps, lnw, ones, eps_sb, tcv, sl, 512)
            _ln_apply(nc, lnw, vecs_sb, tcv, tcv, sl, 512, mu, r,
                      "lnc_g", "lnc_b", fl["lnc_g"], fl["lnc_b"])
        for dt in range(DT):
            nc.scalar.activation(g_sb[dt], tcv[dt], Act.Gelu)
        # x2 = x1 + h2 + gelu(...)  (local cols)
        for dt in range(DT):
            nc.vector.tensor_add(x2_sb[dt], x1_sb[dt][:, 1:TLOC + 1],
                                 h2_sb[dt][:, 1:TLOC + 1])
            nc.vector.tensor_add(x2_sb[dt], x2_sb[dt], g_sb[dt])
    Ps.pop().release()  # cps
    Ls.pop().release()  # conv_t
    Ls.pop().release()  # mid
    if stage == 5:
        return _dbg_exit(x2_sb)

    # ---------------- phase 6: MLP -> output ----------------
    mlpp = tc.alloc_tile_pool(name="mlpp", bufs=1); Ls.append(mlpp)
    h3_sb = [mlpp.tile((128, TLOC), BF16, name=f"h3_{dt}", tag=f"h3_{dt}")
             for dt in range(DT)]
    u_sb = [mlpp.tile((128, TLOC), BF16, name=f"u{jt}", tag=f"u{jt}")
            for jt in range(16)]
    out_sb = [mlpp.tile((128, TLOC), F32, name=f"o{dt}", tag=f"o{dt}")
              for dt in range(DT)]

    w1_sb = []
    for dt in range(DT):
        t = wts.tile((128, DFF), BF16, name=f"w1_{dt}", tag=f"w1_{dt}")
        nc.sync.dma_start(out=t, in_=w1T_d[dt])
        w1_sb.append(t)
    w2_sb = []
    for d2 in range(16):
        t = wts.tile((128, D), BF16, name=f"w2_{d2}", tag=f"w2_{d2}")
        nc.sync.dma_start(out=t, in_=w2T_d[d2])
        w2_sb.append(t)

    lps = tc.alloc_tile_pool(name="lps", bufs=2, space="PSUM"); Ps.append(lps)
    mps = tc.alloc_tile_pool(name="mps", bufs=2, space="PSUM"); Ps.append(mps)
    with nc.named_scope("mlp"):
        for ch in range(2):
            sl = slice(ch * 512, ch * 512 + 512)
            mu, r = _ln_stats(nc, lps, lnw, ones, eps_sb, x2_sb, sl, 512)
            _ln_apply(nc, lnw, vecs_sb, x2_sb, h3_sb, sl, 512, mu, r,
                      "ln3_g", "ln3_b", fl["ln3_g"], fl["ln3_b"])
        for jt in range(16):
            for ch in range(2):
                sl = slice(ch * 512, ch * 512 + 512)
                ps = lps.tile((128, 512), F32, name="ups", tag="ups", bufs=2)
                for dt in range(DT):
                    nc.tensor.matmul(ps, lhsT=w1_sb[dt][:, jt * 128: jt * 128 + 128],
                                     rhs=h3_sb[dt][:, sl],
                                     start=(dt == 0), stop=(dt == DT - 1))
                if fl["b1"]:
                    nc.scalar.activation(u_sb[jt][:, sl], ps, Act.Gelu,
                                         bias=b1_sb[:, jt:jt + 1])
                else:
                    nc.scalar.activation(u_sb[jt][:, sl], ps, Act.Gelu)
        for jt in range(DT):
            for ch in range(2):
                sl = slice(ch * 512, ch * 512 + 512)
                ps = mps.tile((128, 512), F32, name="mmps", tag="m")
                for d2 in range(16):
                    nc.tensor.matmul(ps, lhsT=w2_sb[d2][:, jt * 128: jt * 128 + 128],
                                     rhs=u_sb[d2][:, sl],
                                     start=(d2 == 0), stop=(d2 == 15))
                if fl["b2"]:
                    nc.vector.scalar_tensor_tensor(out=out_sb[jt][:, sl], in0=ps,
                                                   scalar=_vap(vecs_sb, "b2", jt),
                                                   in1=x2_sb[jt][:, sl],
                                                   op0=Alu.add, op1=Alu.add)
                else:
                    nc.vector.tensor_tensor(out_sb[jt][:, sl], ps,
                                            x2_sb[jt][:, sl], Alu.add)
            nc.sync.dma_start(out=yT_d[jt], in_=out_sb[jt])
    Ps.pop().release(); Ps.pop().release()  # mps lps
    Ls.pop().release()  # mlpp
    Ls.pop().release()  # x2p
    Ls.pop().release(); Ls.pop().release(); Ls.pop().release(); Ls.pop().release()
    x1_sb, h2_sb  # keep references


# ======================= host side =======================

def _nz(a):
    return bool(np.any(np.asarray(a) != 0))


def prepare(inputs):
    """Returns (flags, shared_inputs, per_core_inputs[8])."""
    f32 = np.float32
    g = {k: np.asarray(v, f32) for k, v in inputs.items()}
    x = g["x"]
    Wqkv, Wo, W1, W2 = g["Wqkv"], g["Wo"], g["W1"], g["W2"]
    conv_w = g["conv_w"]

    flags = {
        "ln1_g": not np.allclose(g["ln1_g"], 1.0), "ln1_b": _nz(g["ln1_b"]),
        "ln2_g": not np.allclose(g["ln2_g"], 1.0), "ln2_b": _nz(g["ln2_b"]),
        "lnc_g": not np.allclose(g["lnc_g"], 1.0), "lnc_b": _nz(g["lnc_b"]),
        "ln3_g": not np.allclose(g["ln3_g"], 1.0), "ln3_b": _nz(g["ln3_b"]),
        "bq": _nz(g["bqkv"][:D]), "bk": _nz(g["bqkv"][D:2 * D]),
        "cb": _nz(g["conv_b"]),
        "b1": _nz(g["b1"]), "b2": _nz(g["b2"]),
    }
    bv = g["bqkv"][2 * D:]
    bo_eff = g["bo"] + Wo @ bv
    flags["bo"] = _nz(bo_eff)

    bf = ml_dtypes.bfloat16
    shared = {
        "wqkvT": np.ascontiguousarray(Wqkv.T.reshape(DT, 128, 3 * D)).astype(bf),
        "woT": np.ascontiguousarray(Wo.T.reshape(DT, 128, D)).astype(bf),
        "w1T": np.ascontiguousarray(W1.T.reshape(DT, 128, DFF)).astype(bf),
        "w2T": np.ascontiguousarray(W2.T.reshape(16, 128, D)).astype(bf),
        "b1m": np.ascontiguousarray(g["b1"].reshape(16, 128).T).astype(f32),
    }
    vec_vals = {
        "ln1_g": g["ln1_g"], "ln1_b": g["ln1_b"], "ln2_g": g["ln2_g"],
        "ln2_b": g["ln2_b"], "lnc_g": g["lnc_g"], "lnc_b": g["lnc_b"],
        "ln3_g": g["ln3_g"], "ln3_b": g["ln3_b"],
        "cw0": conv_w[:, 0], "cw1": conv_w[:, 1], "cw2": conv_w[:, 2],
        "cb": g["conv_b"], "bo_eff": bo_eff, "bq": g["bqkv"][:D],
        "bk": g["bqkv"][D:2 * D], "b2": g["b2"],
    }
    vecs = np.zeros((128, 4 * len(VEC_NAMES)), f32)
    for i, nme in enumerate(VEC_NAMES):
        vecs[:, 4 * i:4 * i + 4] = vec_vals[nme].reshape(DT, 128).T
    shared["vecs"] = vecs

    per_core = []
    for c in range(NCORES):
        b, half = c // 2, c % 2
        t0 = half * TLOC
        xT = np.ascontiguousarray(x[b].T)                      # (512, 2048)
        xrot = np.roll(xT, -(t0 - 1), axis=1)                  # ext col i = token t0-1+i
        mask = np.ones((128, TEXT), bf)
        if half == 0:
            mask[:, 0] = 0.0
        else:
            mask[:, TEXT - 1] = 0.0
        im = dict(shared)
        im["xT"] = np.ascontiguousarray(xrot.reshape(DT, 128, S)).astype(f32)
        im["mask"] = mask
        per_core.append(im)
    return flags, per_core


_PROG_CACHE = {}


def get_program(flags, stage=6):
    key = (tuple(sorted(flags.items())), stage)
    if key not in _PROG_CACHE:
        _PROG_CACHE[key] = build_program(flags, stage)
    return _PROG_CACHE[key]


def run(inputs, **spmd_kwargs):
    """Run on hardware; returns (output (4,2048,512) f32, BassKernelResults)."""
    flags, per_core = prepare(inputs)
    nc = get_program(flags)
    res = run_bass_kernel_spmd(nc, per_core, core_ids=list(range(NCORES)),
                               **spmd_kwargs)
    out = np.empty((B, S, D), np.float32)
    for c in range(NCORES):
        b, half = c // 2, c % 2
        t0 = half * TLOC
        yT = res.results[c]["yT"].reshape(D, TLOC)
        out[b, t0:t0 + TLOC, :] = yT.T
    return out, res


def kernel(**inputs) -> np.ndarray:
    out, _ = run(inputs)
    return out


def _make_sharded(nc, reps_unused=None):
    import jax
    from jax.sharding import Mesh, PartitionSpec
    from jax.experimental.shard_map import shard_map
    from concourse import bass2jax as b2j
    import concourse.mybir as _mybir

    b2j.install_neuronx_cc_hook()
    fn0 = nc.m.functions[0]
    pid_name = nc.partition_id_tensor.name if nc.partition_id_tensor else None
    in_names, out_names, out_avals, zero_outs = [], [], [], []
    for alloc in fn0.allocations:
        if not isinstance(alloc, _mybir.MemoryLocationSet):
            continue
        name = alloc.memorylocations[0].name
        if alloc.kind == "ExternalInput":
            if name != pid_name:
                in_names.append(name)
        elif alloc.kind == "ExternalOutput":
            out_names.append(name)
            shape = tuple(alloc.tensor_shape)
            dt = _mybir.dt.np(alloc.dtype)
            out_avals.append(jax.core.ShapedArray(shape, dt))
            zero_outs.append(np.zeros(shape, dt))
    n_params = len(in_names)
    all_names = list(in_names) + list(out_names)
    if pid_name is not None:
        all_names.append(pid_name)

    def body(*args):
        operands = list(args)
        if pid_name is not None:
            operands.append(b2j.partition_id_tensor())
        outs = b2j._bass_exec_p.bind(
            *operands,
            out_avals=tuple(out_avals), in_names=tuple(all_names),
            out_names=tuple(out_names), lowering_input_output_aliases=(),
            sim_require_finite=True, sim_require_nnan=True, nc=nc)
        return tuple(outs)

    devices = jax.devices()[:NCORES]
    mesh = Mesh(np.asarray(devices), ("core",))
    P = PartitionSpec
    nin = n_params + len(out_names)
    sharded = jax.jit(shard_map(body, mesh=mesh, in_specs=(P("core"),) * nin,
                                out_specs=(P("core"),) * len(out_names),
                                check_rep=False))
    return sharded, in_names, zero_outs


def _time_dispatch(sharded, concat_in, iters):
    import time as _time
    import jax
    r = sharded(*concat_in)
    jax.block_until_ready(r)
    ts = []
    for _ in range(iters):
        t0 = _time.perf_counter()
        r = sharded(*concat_in)
        jax.block_until_ready(r)
        ts.append(_time.perf_counter() - t0)
    ts.sort()
    return ts[len(ts) // 4]  # lower quartile


def _baseline_nc():
    """Minimal program through the same path, to estimate dispatch overhead."""
    nc = bacc.Bacc("TRN2", target_bir_lowering=False, debug=False)
    xi = nc.dram_tensor("bx", (128, 128), F32, kind="ExternalInput").ap()
    yo = nc.dram_tensor("by", (128, 128), F32, kind="ExternalOutput").ap()
    with tile.TileContext(nc) as tc:
        with tc.tile_pool(name="sb", bufs=1) as sb:
            t = sb.tile((128, 128), F32, name="bt", tag="bt")
            nc.sync.dma_start(out=t, in_=xi)
            nc.sync.dma_start(out=yo, in_=t)
    nc.compile()
    return nc


def timed_run(inputs, reps=30, batches=3):
    """Estimate on-device exec time: single-dispatch wall time minus the
    dispatch overhead of a minimal kernel through the same path."""
    flags, per_core = prepare(inputs)
    nc = get_program(flags)
    sharded, in_names, zero_outs = _make_sharded(nc)
    concat_in = [np.concatenate([np.asarray(per_core[c][nm]) for c in range(NCORES)],
                                axis=0) for nm in in_names]
    concat_in += [np.concatenate([z] * NCORES, axis=0) for z in zero_outs]
    t_full = _time_dispatch(sharded, concat_in, reps)

    print(f"  dispatch(full)={t_full*1e6:.0f}us (upper bound incl. host dispatch)")
    return t_full * 1e9


def kernel(**inputs) -> np.ndarray:
    out, _ = run(inputs)
    return out


def timed_run(inputs, reps=30, batches=3):
    """Time repeated on-device executes of the compiled program (test helper).

    Replicates bass2jax.run_bass_via_pjrt's multi-core path, but keeps inputs
    device-resident and chains `reps` sequential executes inside one jit (a
    zero-valued scalar from each iteration's output is added to a small input
    of the next to prevent CSE/reordering). Returns best per-iteration ns.
    """
    import time as _time
    import jax
    from jax.sharding import Mesh, PartitionSpec
    from jax.experimental.shard_map import shard_map
    from concourse import bass2jax as b2j
    import concourse.mybir as _mybir

    flags, per_core = prepare(inputs)
    nc = get_program(flags)
    b2j.install_neuronx_cc_hook()

    fn0 = nc.m.functions[0]
    pid_name = nc.partition_id_tensor.name if nc.partition_id_tensor else None
    in_names, out_names, out_avals, zero_outs = [], [], [], []
    for alloc in fn0.allocations:
        if not isinstance(alloc, _mybir.MemoryLocationSet):
            continue
        name = alloc.memorylocations[0].name
        if alloc.kind == "ExternalInput":
            if name != pid_name:
                in_names.append(name)
        elif alloc.kind == "ExternalOutput":
            out_names.append(name)
            shape = tuple(alloc.tensor_shape)
            dt = _mybir.dt.np(alloc.dtype)
            out_avals.append(jax.core.ShapedArray(shape, dt))
            zero_outs.append(np.zeros(shape, dt))
    n_params = len(in_names)
    all_names = tuple(in_names + out_names)
    vidx = in_names.index("vecs")

    if pid_name is not None:
        all_names = tuple(list(all_names) + [pid_name])

    def body(*args):
        arrs = list(args[:n_params])
        zeros = list(args[n_params:])
        outs = None
        for _ in range(reps):
            operands = arrs + zeros
            if pid_name is not None:
                operands = operands + [b2j.partition_id_tensor()]
            outs = b2j._bass_exec_p.bind(
                *operands,
                out_avals=tuple(out_avals), in_names=all_names,
                out_names=tuple(out_names), lowering_input_output_aliases=(),
                sim_require_finite=True, sim_require_nnan=True, nc=nc)
            arrs[vidx] = arrs[vidx] + outs[0].reshape(-1)[0] * 0.0
        return tuple(outs)

    devices = jax.devices()[:NCORES]
    mesh = Mesh(np.asarray(devices), ("core",))
    P = PartitionSpec
    nin = n_params + len(out_names)
    sharded = jax.jit(shard_map(body, mesh=mesh, in_specs=(P("core"),) * nin,
                                out_specs=(P("core"),) * len(out_names),
                                check_rep=False))
    concat_in = [np.concatenate([np.asarray(per_core[c][nm]) for c in range(NCORES)], axis=0)
                 for nm in in_names]
    concat_in += [np.concatenate([z] * NCORES, axis=0) for z in zero_outs]
    r = sharded(*concat_in)
    jax.block_until_ready(r)
    best = float("inf")
    for _ in range(batches):
        t0 = _time.perf_counter()
        r = sharded(*concat_in)
        jax.block_until_ready(r)
        dt_s = _time.perf_counter() - t0
        best = min(best, dt_s / reps)
    return best * 1e9


# revision 33
# speedup vs baseline: 1.0254x; 1.0254x over previous
"""Trainium2 Bass kernel for an enhanced transformer block (attn + depthwise-conv + MLP).

Sharding: 8 cores = 4 batches x 2 sequence halves (data parallel, no collectives).
Each core receives its batch's x TRANSPOSED (feature-major: d on partitions,
tokens on the free axis) and ROTATED so that its extended token range
[t0-1, t1+1) lands at columns [0, 1026) uniformly on every core (SPMD: one
program, different data). K/V are computed over the full (rotated) sequence;
q/attention only over the core's 1026 extended columns. The rotation makes
attention sums run over a permuted key order, which is mathematically
identical. Halo columns provide the depthwise-conv neighbor values; at
sequence edges the halo is dead (wrapped garbage) and is zeroed via a mask
folded into LN2's rstd.

Softmax is computed without max-subtraction (scores for this problem are
O(1); exp cannot overflow) so the denominator can be accumulated by an
extra all-ones column appended to V in the P@V matmul.
"""

import numpy as np
import ml_dtypes

import concourse.bass as bass
import concourse.bacc as bacc
import concourse.mybir as mybir
import concourse.tile as tile
from concourse.bass_utils import run_bass_kernel_spmd

F32 = mybir.dt.float32
F32R = mybir.dt.float32r
BF16 = mybir.dt.bfloat16
Alu = mybir.AluOpType
Act = mybir.ActivationFunctionType

D = 512          # model dim
S = 2048         # sequence length
B = 4            # batch
H = 8            # heads
HD = 64          # head dim
DFF = 2048       # mlp hidden
NCORES = 8
TLOC = 1024      # local tokens per core
TEXT = 1026      # extended (1 halo col each side)
DT = 4           # d-tiles of 128
EPS = 1e-5

# order of packed 512-length vectors in the "vecs" input
VEC_NAMES = ["ln1_g", "ln1_b", "ln2_g", "ln2_b", "lnc_g", "lnc_b",
             "ln3_g", "ln3_b", "cw0", "cw1", "cw2", "cb",
             "bo_eff", "bq", "bk", "b2"]
VIDX = {n: i for i, n in enumerate(VEC_NAMES)}


def _vap(vecs_sb, name, dt):
    """per-partition [128,1] scalar AP for vector `name`, d-tile dt."""
    c = 4 * VIDX[name] + dt
    return vecs_sb[:, c:c + 1]


def build_program(flags, stage=6):
    """Trace the uniform per-core program. flags: dict of bools enabling
    optional bias/scale terms (specialized to the actual input values).
    stage<6 emits an intermediate tensor and stops (debug bisection)."""
    nc = bacc.Bacc("TRN2", target_bir_lowering=False, debug=False)

    xT_d = nc.dram_tensor("xT", (DT, 128, S), F32, kind="ExternalInput").ap()
    wqkvT_d = nc.dram_tensor("wqkvT", (DT, 128, 3 * D), BF16, kind="ExternalInput").ap()
    woT_d = nc.dram_tensor("woT", (DT, 128, D), BF16, kind="ExternalInput").ap()
    w1T_d = nc.dram_tensor("w1T", (DT, 128, DFF), BF16, kind="ExternalInput").ap()
    w2T_d = nc.dram_tensor("w2T", (16, 128, D), BF16, kind="ExternalInput").ap()
    vecs_d = nc.dram_tensor("vecs", (128, 4 * len(VEC_NAMES)), F32, kind="ExternalInput").ap()
    b1m_d = nc.dram_tensor("b1m", (128, 16), F32, kind="ExternalInput").ap()
    mask_d = nc.dram_tensor("mask", (128, TEXT), BF16, kind="ExternalInput").ap()
    yT_d = nc.dram_tensor("yT", (DT, 128, TLOC), F32, kind="ExternalOutput").ap()

    with tile.TileContext(nc) as tc:
        _prog(nc, tc, flags,
              xT_d, wqkvT_d, woT_d, w1T_d, w2T_d, vecs_d, b1m_d, mask_d, yT_d,
              stage=stage)
    nc.compile()
    return nc


def _ln_stats(nc, lnps, lnw, ones, eps_sb, z_tiles, sl, n):
    """LN stats over the d axis (partitions x 4 tiles) for token cols `sl`
    (length n). Returns (mu_rep, r_rep) fp32 SBUF tiles (128, n), replicated
    across partitions. z_tiles: 4 fp32 SBUF tiles (128, >=n cols)."""
    s1 = lnps.tile((128, 512), F32, name="s1", tag="s1", bufs=2)
    s2 = lnps.tile((128, 512), F32, name="s2", tag="s2", bufs=2)
    for dt in range(DT):
        xb = lnw.tile((128, 512), BF16, name="xb", tag="xb", bufs=4)
        nc.vector.tensor_copy(xb[:, :n], z_tiles[dt][:, sl])
        nc.tensor.matmul(s1[:, :n], lhsT=ones, rhs=xb[:, :n],
                         start=(dt == 0), stop=(dt == DT - 1))
        sq = lnw.tile((128, 512), BF16, name="sq", tag="sq", bufs=4)
        nc.scalar.square(sq[:, :n], z_tiles[dt][:, sl])
        nc.tensor.matmul(s2[:, :n], lhsT=ones, rhs=sq[:, :n],
                         start=(dt == 0), stop=(dt == DT - 1))
    mu = lnw.tile((128, 512), F32, name="mu", tag="mu")
    nc.vector.tensor_scalar_mul(out=mu[:, :n], in0=s1[:, :n], scalar1=1.0 / D)
    mu2 = lnw.tile((128, 512), F32, name="mu2", tag="scratch", bufs=3)
    nc.vector.tensor_mul(mu2[:, :n], mu[:, :n], mu[:, :n])
    m2s = lnw.tile((128, 512), F32, name="m2s", tag="scratch", bufs=3)
    nc.vector.tensor_scalar_mul(out=m2s[:, :n], in0=s2[:, :n], scalar1=1.0 / D)
    var = lnw.tile((128, 512), F32, name="var", tag="var")
    nc.vector.tensor_sub(var[:, :n], m2s[:, :n], mu2[:, :n])
    sd = lnw.tile((128, 512), F32, name="sd", tag="scratch", bufs=3)
    nc.scalar.activation(sd[:, :n], var[:, :n], Act.Sqrt, bias=eps_sb[:, 0:1])
    r = lnw.tile((128, 512), F32, name="r", tag="r")
    nc.vector.reciprocal(r[:, :n], sd[:, :n])
    return mu, r


def _ln_apply(nc, lnw, vecs_sb, z_tiles, out_tiles, sl, n, mu, r,
              gname, bname, gflag, bflag, out_sl=None):
    """out = (z - mu) * r [* g] [+ b] for each d-tile, cols sl."""
    osl = sl if out_sl is None else out_sl
    for dt in range(DT):
        xc = lnw.tile((128, 512), F32, name="xc", tag="xc", bufs=2)
        nc.vector.tensor_sub(xc[:, :n], z_tiles[dt][:, sl], mu[:, :n])
        dst = out_tiles[dt][:, osl]
        if gflag:
            nc.vector.scalar_tensor_tensor(out=dst, in0=xc[:, :n],
                                           scalar=_vap(vecs_sb, gname, dt),
                                           in1=r[:, :n], op0=Alu.mult, op1=Alu.mult)
        else:
            nc.vector.tensor_mul(dst, xc[:, :n], r[:, :n])
        if bflag:
            nc.vector.tensor_scalar_add(out=dst, in0=dst,
                                        scalar1=_vap(vecs_sb, bname, dt))


def _prog(nc, tc, fl, xT_d, wqkvT_d, woT_d, w1T_d, w2T_d, vecs_d, b1m_d,
          mask_d, yT_d, stage=6):
    Ls, Rs, Ps = [], [], []  # open-pool stacks (left / right / psum)

    def _dbg_exit(tiles):
        dbg = tc.alloc_tile_pool(name="dbgout", bufs=1)
        for dt in range(DT):
            t = dbg.tile((128, TLOC), F32, name=f"dbg{dt}", tag=f"dbg{dt}")
            nc.vector.tensor_copy(t, tiles[dt][:, 0:TLOC])
            nc.sync.dma_start(out=yT_d[dt], in_=t)
        dbg.release()
        for st in (Ps, Ls, Rs):
            while st:
                st.pop().release()

    # ---------------- persistent pools ----------------
    consts = tc.alloc_tile_pool(name="consts", bufs=1); Ls.append(consts)
    wts = tc.alloc_tile_pool(name="wts", bufs=1); Ls.append(wts)
    lnw = tc.alloc_tile_pool(name="lnw", bufs=2); Ls.append(lnw)
    small = tc.alloc_tile_pool(name="small", bufs=2); Ls.append(small)

    vecs_sb = consts.tile((128, 4 * len(VEC_NAMES)), F32, name="vecs_sb", tag="vecs")
    nc.sync.dma_start(out=vecs_sb, in_=vecs_d)
    b1_sb = consts.tile((128, 16), F32, name="b1_sb", tag="b1")
    nc.sync.dma_start(out=b1_sb, in_=b1m_d)
    mask_sb = consts.tile((128, TEXT), BF16, name="mask_sb", tag="mask")
    nc.sync.dma_start(out=mask_sb, in_=mask_d)
    ones = consts.tile((128, 128), BF16, name="ones", tag="ones")
    nc.vector.memset(ones, 1.0)
    eps_sb = consts.tile((128, 1), F32, name="eps_sb", tag="eps")
    nc.vector.memset(eps_sb, EPS)

    wqkv_sb = []
    for dt in range(DT):
        t = wts.tile((128, 3 * D), BF16, name=f"wqkv{dt}", tag=f"wqkv{dt}")
        nc.sync.dma_start(out=t, in_=wqkvT_d[dt])
        wqkv_sb.append(t)
    wo_sb = []
    for dt in range(DT):
        t = wts.tile((128, D), BF16, name=f"wo{dt}", tag=f"wo{dt}")
        nc.sync.dma_start(out=t, in_=woT_d[dt])
        wo_sb.append(t)

    # x_res: residual slice of x (cols 0:TEXT), outlives the full-x tiles
    xres_pool = tc.alloc_tile_pool(name="xres_pool", bufs=1, side="right"); Rs.append(xres_pool)
    xres_sb = [xres_pool.tile((128, TEXT), F32, name=f"xr{dt}", tag=f"xr{dt}")
               for dt in range(DT)]
    # aT (attention output, feature-major) - lives until out-proj
    a_pool = tc.alloc_tile_pool(name="a_pool", bufs=1, side="right"); Rs.append(a_pool)
    a_sb = [a_pool.tile((128, TEXT), BF16, name=f"a{dt}", tag=f"a{dt}")
            for dt in range(DT)]
    # k/v/q - live until end of attention
    kvq = tc.alloc_tile_pool(name="kvq", bufs=1, side="right"); Rs.append(kvq)

    # hT (LN1 output, bf16) - lives until end of QKV
    h_pool = tc.alloc_tile_pool(name="h_pool", bufs=1); Ls.append(h_pool)
    h_sb = [h_pool.tile((128, S), BF16, name=f"h{dt}", tag=f"h{dt}")
            for dt in range(DT)]

    # x tiles (feature-major, rotated), full sequence
    x_pool = tc.alloc_tile_pool(name="x_pool", bufs=1); Ls.append(x_pool)
    x_sb = []
    for dt in range(DT):
        t = x_pool.tile((128, S), F32, name=f"x{dt}", tag=f"x{dt}")
        nc.sync.dma_start(out=t, in_=xT_d[dt])
        x_sb.append(t)

    # ---------------- phase 1: LN1 over full sequence -> hT (bf16) --------
    ln1ps = tc.alloc_tile_pool(name="ln1ps", bufs=2, space="PSUM"); Ps.append(ln1ps)
    with nc.named_scope("ln1"):
        for ch in range(4):
            sl = slice(ch * 512, ch * 512 + 512)
            mu, r = _ln_stats(nc, ln1ps, lnw, ones, eps_sb, x_sb, sl, 512)
            _ln_apply(nc, lnw, vecs_sb, x_sb, h_sb, sl, 512, mu, r,
                      "ln1_g", "ln1_b", fl["ln1_g"], fl["ln1_b"])
    Ps.pop().release()
    for dt in range(DT):
        nc.vector.tensor_copy(xres_sb[dt], x_sb[dt][:, 0:TEXT])
    Ls.pop().release()  # x_pool
    if stage == 1:
        return _dbg_exit(h_sb)

    # ---------------- phase 2: QKV ----------------
    k_sb = [kvq.tile((128, S), BF16, name=f"k{dt}", tag=f"k{dt}") for dt in range(DT)]
    v_sb = [kvq.tile((128, H, HD + 1), BF16, name=f"v{tc_}", tag=f"v{tc_}")
            for tc_ in range(16)]
    q_sb = [kvq.tile((128, TEXT), BF16, name=f"q{dt}", tag=f"q{dt}")
            for dt in range(DT)]

    qkvps = tc.alloc_tile_pool(name="qkvps", bufs=4, space="PSUM"); Ps.append(qkvps)
    with nc.named_scope("qkv"):
        # k: feature-major (j on partitions, tokens free)
        for jt in range(DT):
            for ch in range(4):
                sl = slice(ch * 512, ch * 512 + 512)
                ps = qkvps.tile((128, 512), F32, name="kps", tag="mm")
                for dt in range(DT):
                    nc.tensor.matmul(ps, lhsT=wqkv_sb[dt][:, D + jt * 128: D + jt * 128 + 128],
                                     rhs=h_sb[dt][:, sl],
                                     start=(dt == 0), stop=(dt == DT - 1))
                if fl["bk"]:
                    nc.scalar.add(out=k_sb[jt][:, sl], in_=ps,
                                  add=_vap(vecs_sb, "bk", jt))
                else:
                    nc.scalar.copy(k_sb[jt][:, sl], ps)
        # q: feature-major, extended token range only
        for jt in range(DT):
            for (c0, n) in ((0, 512), (512, 512), (1024, 2)):
                tag = "mm" if n == 512 else "qtiny"
                ps = qkvps.tile((128, 512) if n == 512 else (128, 2), F32,
                                name="qps", tag=tag, bufs=4 if n == 512 else 2)
                for dt in range(DT):
                    nc.tensor.matmul(ps[:, :n], lhsT=wqkv_sb[dt][:, jt * 128: jt * 128 + 128],
                                     rhs=h_sb[dt][:, c0:c0 + n],
                                     start=(dt == 0), stop=(dt == DT - 1))
                if fl["bq"]:
                    nc.scalar.add(out=q_sb[jt][:, c0:c0 + n], in_=ps[:, :n],
                                  add=_vap(vecs_sb, "bq", jt))
                else:
                    nc.scalar.copy(q_sb[jt][:, c0:c0 + n], ps[:, :n])
        # v: token-major (tokens on partitions, j free), with ones column
        for tc_ in range(16):
            nc.vector.memset(v_sb[tc_][:, :, HD:HD + 1], 1.0)
            ps = qkvps.tile((128, 512), F32, name="vps", tag="mm")
            for dt in range(DT):
                nc.tensor.matmul(ps, lhsT=h_sb[dt][:, tc_ * 128: tc_ * 128 + 128],
                                 rhs=wqkv_sb[dt][:, 2 * D:3 * D],
                                 start=(dt == 0), stop=(dt == DT - 1))
            src = ps[:, :].rearrange("p (h d) -> p h d", h=H)
            # v bias would be per-free here; it is folded into bo_eff on host.
            nc.scalar.copy(v_sb[tc_][:, :, 0:HD], src)
    Ps.pop().release()  # qkvps
    Ls.pop().release()  # h_pool
    if stage == 2:
        return _dbg_exit(k_sb)

    # ---------------- phase 3: attention ----------------
    p_pool = tc.alloc_tile_pool(name="p_pool", bufs=4, side="right"); Rs.append(p_pool)
    scps = tc.alloc_tile_pool(name="scps", bufs=4, space="PSUM"); Ps.append(scps)
    avps = tc.alloc_tile_pool(name="avps", bufs=2, space="PSUM"); Ps.append(avps)

    with nc.named_scope("attn"):
        for hp in range(4):  # head pairs: a=2hp (rows 0:64), b=2hp+1 (rows 64:128)
            av_ab = [avps.tile((128, 1024), F32, name=f"av{hp}_{i}", tag="av")
                     for i in range(2)]
            rows = [slice(0, 64), slice(64, 128)]
            for kc in range(16):
                ksl = slice(kc * 128, kc * 128 + 128)
                ptiles = [None, None]
                for i in range(2):
                    sc = scps.tile((128, 1024), F32, name="sc", tag="sc", bufs=2)
                    for qc in range(2):
                        nc.tensor.matmul(sc[:, qc * 512:(qc + 1) * 512],
                                         lhsT=k_sb[hp][rows[i], ksl],
                                         rhs=q_sb[hp][rows[i], qc * 512:(qc + 1) * 512],
                                         start=True, stop=True)
                    pt = p_pool.tile((128, 1024), BF16, name="pt", tag="pt")
                    nc.scalar.activation(pt, sc, Act.Exp, scale=0.125)
                    ptiles[i] = pt
                # av accumulation
                for i in range(2):
                    for qc in range(2):
                        nc.tensor.matmul(av_ab[i][0:HD + 1, qc * 512:(qc + 1) * 512],
                                         lhsT=v_sb[kc][:, 2 * hp + i, :],
                                         rhs=ptiles[i][:, qc * 512:(qc + 1) * 512],
                                         start=(kc == 0), stop=(kc == 15))
            # normalize: recip of denominator row, replicate via K=1 matmul,
            # stage to SBUF (DVE reads only one PSUM operand), multiply
            for i in range(2):
                if stage == 31:
                    nc.vector.tensor_copy(a_sb[hp][rows[i], 0:1024],
                                          av_ab[i][0:64, :])
                    continue
                rec = small.tile((1, 1024), BF16, name="rec", tag="rec")
                with nc.allow_low_precision("bf16 softmax denom recip (attn out is tiny)"):
                    nc.vector.reciprocal(rec, av_ab[i][HD:HD + 1, :])
                for qc in range(2):
                    qsl = slice(qc * 512, qc * 512 + 512)
                    nc.tensor.matmul(av_ab[i][64:128, qsl],
                                     lhsT=ones[0:1, 0:64], rhs=rec[:, qsl],
                                     start=True, stop=True)
                rrep = small.tile((64, 1024), BF16, name="rrep", tag="rrep")
                nc.vector.tensor_copy(rrep, av_ab[i][64:128, :])
                nc.vector.tensor_tensor(a_sb[hp][rows[i], 0:1024],
                                        av_ab[i][0:64, :], rrep,
                                        Alu.mult)
    Ps.pop().release(); Ps.pop().release()  # avps scps
    Rs.pop().release()  # p_pool
    if stage in (3, 31, 32):
        Rs.pop().release()  # kvq
        return _dbg_exit(a_sb)

    # ---------------- phase 4: out-proj + residual -> x1 ----------------
    x2p = tc.alloc_tile_pool(name="x2p", bufs=1); Ls.append(x2p)
    x2_sb = [x2p.tile((128, TLOC), F32, name=f"x2_{dt}", tag=f"x2_{dt}")
             for dt in range(DT)]
    mid = tc.alloc_tile_pool(name="mid", bufs=1); Ls.append(mid)
    x1_sb = [mid.tile((128, TEXT), F32, name=f"x1_{dt}", tag=f"x1_{dt}")
             for dt in range(DT)]
    ops = tc.alloc_tile_pool(name="ops", bufs=3, space="PSUM"); Ps.append(ops)
    QC3 = ((0, 342), (342, 342), (684, 342))
    # -- halo attention (2 ext cols per core), token-major scores --
    phd_d = nc.dram_tensor("phd_scratch", (H, 2, S), BF16).ap()
    dsum_d = nc.dram_tensor("dsum_scratch", (H, 2, 1), F32).ap()
    hps = tc.alloc_tile_pool(name="hps", bufs=1, space="PSUM"); Ps.append(hps)
    hsb = tc.alloc_tile_pool(name="hsb", bufs=2)
    with nc.named_scope("halo"):
        for h in range(H):
            hp, i = h // 2, h % 2
            rws = slice(64 * i, 64 * i + 64)
            ph = hsb.tile((2, S), BF16, name="ph", tag="ph", bufs=1)
            dsum = hsb.tile((2, 2), F32, name="dsum", tag="dsum")
            for c2 in range(2):
                sch = hps.tile((2, 1024), F32, name="sch", tag="sch", bufs=1)
                for c in range(2):
                    cc = 2 * c2 + c
                    nc.tensor.matmul(sch[:, c * 512:(c + 1) * 512],
                                     lhsT=q_sb[hp][rws, 1024:1026],
                                     rhs=k_sb[hp][rws, cc * 512:(cc + 1) * 512],
                                     start=True, stop=True)
                nc.scalar.activation(ph[:, c2 * 1024:(c2 + 1) * 1024], sch,
                                     Act.Exp, scale=0.125,
                                     accum_out=dsum[:, c2:c2 + 1])
            nc.vector.tensor_add(dsum[:, 0:1], dsum[:, 0:1], dsum[:, 1:2])
            nc.sync.dma_start(out=phd_d[h], in_=ph)
            nc.sync.dma_start(out=dsum_d[h], in_=dsum[:, 0:1])
            pT = hsb.tile((128, 16, 2), BF16, name="pT", tag="pT")
            for q in range(2):
                nc.sync.dma_start(out=pT[:, :, q],
                                  in_=phd_d[h][q].rearrange("(c p) -> p c", p=128))
            denT = hsb.tile((1, 2), F32, name="denT", tag="denT")
            nc.sync.dma_start(out=denT, in_=dsum_d[h].rearrange("q one -> one q"))
            avh = hps.tile((128, 2), F32, name="avh", tag="avh", bufs=2)
            for kc in range(16):
                nc.tensor.matmul(avh[0:64, :], lhsT=v_sb[kc][:, h, 0:HD],
                                 rhs=pT[:, kc, :], start=(kc == 0), stop=(kc == 15))
            rec2 = hsb.tile((1, 2), BF16, name="rec2", tag="rec2")
            with nc.allow_low_precision("bf16 halo softmax recip"):
                nc.vector.reciprocal(rec2, denT)
            nc.tensor.matmul(avh[64:128, :], lhsT=ones[0:1, 0:64], rhs=rec2,
                             start=True, stop=True)
            rr2 = hsb.tile((64, 2), BF16, name="rr2", tag="rr2")
            nc.vector.tensor_copy(rr2, avh[64:128, :])
            nc.vector.tensor_tensor(a_sb[hp][rws, 1024:1026], avh[0:64, :],
                                    rr2, Alu.mult)
    hsb.release()
    Ps.pop().release()  # hps
    Rs.pop().release()  # kvq
    with nc.named_scope("outproj"):
        for jt in range(DT):
            for (c0, n) in QC3:
                sl = slice(c0, c0 + n)
                ps = ops.tile((128, 342), F32, name="ops_t", tag="o")
                for dt in range(DT):
                    nc.tensor.matmul(ps[:, :n], lhsT=wo_sb[dt][:, jt * 128: jt * 128 + 128],
                                     rhs=a_sb[dt][:, sl],
                                     start=(dt == 0), stop=(dt == DT - 1))
                if fl["bo"]:
                    nc.vector.scalar_tensor_tensor(out=x1_sb[jt][:, sl], in0=ps[:, :n],
                                                   scalar=_vap(vecs_sb, "bo_eff", jt),
                                                   in1=xres_sb[jt][:, sl],
                                                   op0=Alu.add, op1=Alu.add)
                else:
                    nc.vector.tensor_tensor(x1_sb[jt][:, sl], ps[:, :n],
                                            xres_sb[jt][:, sl], Alu.add)
    Ps.pop().release()  # ops
    Rs.pop().release()  # a_pool
    Rs.pop().release()  # xres_pool
    if stage == 4:
        return _dbg_exit(x1_sb)

    # ---------------- phase 5: conv block -> x2 ----------------
    h2_sb = [mid.tile((128, TEXT), F32, name=f"h2_{dt}", tag=f"h2_{dt}")
             for dt in range(DT)]
    conv_t = tc.alloc_tile_pool(name="conv_t", bufs=1); Ls.append(conv_t)
    tcv = [conv_t.tile((128, TLOC), F32, name=f"tc{dt}", tag=f"tc{dt}")
           for dt in range(DT)]
    g_sb = [conv_t.tile((128, TLOC), F32, name=f"g{dt}", tag=f"g{dt}")
            for dt in range(DT)]

    cps = tc.alloc_tile_pool(name="cps", bufs=2, space="PSUM"); Ps.append(cps)
    with nc.named_scope("convblock"):
        # LN2 over 1026 cols (3 chunks of 342), rstd masked at dead halo cols
        for (c0, n) in QC3:
            sl = slice(c0, c0 + n)
            mu, r = _ln_stats(nc, cps, lnw, ones, eps_sb, x1_sb, sl, n)
            nc.vector.tensor_mul(r[:, :n], r[:, :n], mask_sb[:, sl])
            _ln_apply(nc, lnw, vecs_sb, x1_sb, h2_sb, sl, n, mu, r,
                      "ln2_g", "ln2_b", fl["ln2_g"], fl["ln2_b"])
        # depthwise conv along tokens (output = local cols [1,1025) -> 1024)
        for dt in range(DT):
            tmp = conv_t.tile((128, TLOC), F32, name="ctmp", tag="ctmp", bufs=2)
            if fl["cb"]:
                nc.vector.tensor_scalar(out=tmp, in0=h2_sb[dt][:, 0:TLOC],
                                        scalar1=_vap(vecs_sb, "cw0", dt),
                                        scalar2=_vap(vecs_sb, "cb", dt),
                                        op0=Alu.mult, op1=Alu.add)
            else:
                nc.vector.tensor_scalar_mul(out=tmp, in0=h2_sb[dt][:, 0:TLOC],
                                            scalar1=_vap(vecs_sb, "cw0", dt))
            nc.vector.scalar_tensor_tensor(out=tmp, in0=h2_sb[dt][:, 1:TLOC + 1],
                                           scalar=_vap(vecs_sb, "cw1", dt),
                                           in1=tmp, op0=Alu.mult, op1=Alu.add)
            nc.vector.scalar_tensor_tensor(out=tcv[dt], in0=h2_sb[dt][:, 2:TLOC + 2],
                                           scalar=_vap(vecs_sb, "cw2", dt),
                                           in1=tmp, op0=Alu.mult, op1=Alu.add)
        # LNc on conv output (local 1024), then gelu
        for ch in range(2):
            sl = slice(ch * 512, ch * 512 + 512)
            mu, r = _ln_stats(nc, cps, lnw, ones, eps_sb, tcv, sl, 512)
            _ln_apply(nc, lnw, vecs_sb, tcv, tcv, sl, 512, mu, r,
                      "lnc_g", "lnc_b", fl["lnc_g"], fl["lnc_b"])
        for dt in range(DT):
            nc.scalar.activation(g_sb[dt], tcv[dt], Act.Gelu)
        # x2 = x1 + h2 + gelu(...)  (local cols)
        for dt in range(DT):
            nc.vector.tensor_add(x2_sb[dt], x1_sb[dt][:, 1:TLOC + 1],
                                 h2_sb[dt][:, 1:TLOC + 1])
            nc.vector.tensor_add(x2_sb[dt], x2_sb[dt], g_sb[dt])
    Ps.pop().release()  # cps
    Ls.pop().release()  # conv_t
    Ls.pop().release()  # mid
    if stage == 5:
        return _dbg_exit(x2_sb)

    # ---------------- phase 6: MLP -> output ----------------
    mlpp = tc.alloc_tile_pool(name="mlpp", bufs=1); Ls.append(mlpp)
    h3_sb = [mlpp.tile((128, TLOC), BF16, name=f"h3_{dt}", tag=f"h3_{dt}")
             for dt in range(DT)]
    u_sb = [mlpp.tile((128, TLOC), BF16, name=f"u{jt}", tag=f"u{jt}")
            for jt in range(16)]
    out_sb = [mlpp.tile((128, TLOC), F32, name=f"o{dt}", tag=f"o{dt}")
              for dt in range(DT)]

    w1_sb = []
    for dt in range(DT):
        t = wts.tile((128, DFF), BF16, name=f"w1_{dt}", tag=f"w1_{dt}")
        nc.sync.dma_start(out=t, in_=w1T_d[dt])
        w1_sb.append(t)
    w2_sb = []
    for d2 in range(16):
        t = wts.tile((128, D), BF16, name=f"w2_{d2}", tag=f"w2_{d2}")
        nc.sync.dma_start(out=t, in_=w2T_d[d2])
        w2_sb.append(t)

    lps = tc.alloc_tile_pool(name="lps", bufs=2, space="PSUM"); Ps.append(lps)
    mps = tc.alloc_tile_pool(name="mps", bufs=2, space="PSUM"); Ps.append(mps)
    with nc.named_scope("mlp"):
        for ch in range(2):
            sl = slice(ch * 512, ch * 512 + 512)
            mu, r = _ln_stats(nc, lps, lnw, ones, eps_sb, x2_sb, sl, 512)
            _ln_apply(nc, lnw, vecs_sb, x2_sb, h3_sb, sl, 512, mu, r,
                      "ln3_g", "ln3_b", fl["ln3_g"], fl["ln3_b"])
        for jt in range(16):
            for ch in range(2):
                sl = slice(ch * 512, ch * 512 + 512)
                ps = lps.tile((128, 512), F32, name="ups", tag="ups", bufs=2)
                for dt in range(DT):
                    nc.tensor.matmul(ps, lhsT=w1_sb[dt][:, jt * 128: jt * 128 + 128],
                                     rhs=h3_sb[dt][:, sl],
                                     start=(dt == 0), stop=(dt == DT - 1))
                if fl["b1"]:
                    nc.scalar.activation(u_sb[jt][:, sl], ps, Act.Gelu,
                                         bias=b1_sb[:, jt:jt + 1])
                else:
                    nc.scalar.activation(u_sb[jt][:, sl], ps, Act.Gelu)
        for jt in range(DT):
            for ch in range(2):
                sl = slice(ch * 512, ch * 512 + 512)
                ps = mps.tile((128, 512), F32, name="mmps", tag="m")
                for d2 in range(16):
                    nc.tensor.matmul(ps, lhsT=w2_sb[d2][:, jt * 128: jt * 128 + 128],
                                     rhs=u_sb[d2][:, sl],
                                     start=(d2 == 0), stop=(d2 == 15))
                if fl["b2"]:
                    nc.vector.scalar_tensor_tensor(out=out_sb[jt][:, sl], in0=ps,
                                                   scalar=_vap(vecs_sb, "b2", jt),
                                                   in1=x2_sb[jt][:, sl],
                                                   op0=Alu.add, op1=Alu.add)
                else:
                    nc.vector.tensor_tensor(out_sb[jt][:, sl], ps,
                                            x2_sb[jt][:, sl], Alu.add)
            nc.sync.dma_start(out=yT_d[jt], in_=out_sb[jt])
    Ps.pop().release(); Ps.pop().release()  # mps lps
    Ls.pop().release()  # mlpp
    Ls.pop().release()  # x2p
    Ls.pop().release(); Ls.pop().release(); Ls.pop().release(); Ls.pop().release()
    x1_sb, h2_sb  # keep references


# ======================= host side =======================

def _nz(a):
    return bool(np.any(np.asarray(a) != 0))


def prepare(inputs):
    """Returns (flags, shared_inputs, per_core_inputs[8])."""
    f32 = np.float32
    g = {k: np.asarray(v, f32) for k, v in inputs.items()}
    x = g["x"]
    Wqkv, Wo, W1, W2 = g["Wqkv"], g["Wo"], g["W1"], g["W2"]
    conv_w = g["conv_w"]

    flags = {
        "ln1_g": not np.allclose(g["ln1_g"], 1.0), "ln1_b": _nz(g["ln1_b"]),
        "ln2_g": not np.allclose(g["ln2_g"], 1.0), "ln2_b": _nz(g["ln2_b"]),
        "lnc_g": not np.allclose(g["lnc_g"], 1.0), "lnc_b": _nz(g["lnc_b"]),
        "ln3_g": not np.allclose(g["ln3_g"], 1.0), "ln3_b": _nz(g["ln3_b"]),
        "bq": _nz(g["bqkv"][:D]), "bk": _nz(g["bqkv"][D:2 * D]),
        "cb": _nz(g["conv_b"]),
        "b1": _nz(g["b1"]), "b2": _nz(g["b2"]),
    }
    bv = g["bqkv"][2 * D:]
    bo_eff = g["bo"] + Wo @ bv
    flags["bo"] = _nz(bo_eff)

    bf = ml_dtypes.bfloat16
    shared = {
        "wqkvT": np.ascontiguousarray(Wqkv.T.reshape(DT, 128, 3 * D)).astype(bf),
        "woT": np.ascontiguousarray(Wo.T.reshape(DT, 128, D)).astype(bf),
        "w1T": np.ascontiguousarray(W1.T.reshape(DT, 128, DFF)).astype(bf),
        "w2T": np.ascontiguousarray(W2.T.reshape(16, 128, D)).astype(bf),
        "b1m": np.ascontiguousarray(g["b1"].reshape(16, 128).T).astype(f32),
    }
    vec_vals = {
        "ln1_g": g["ln1_g"], "ln1_b": g["ln1_b"], "ln2_g": g["ln2_g"],
        "ln2_b": g["ln2_b"], "lnc_g": g["lnc_g"], "lnc_b": g["lnc_b"],
        "ln3_g": g["ln3_g"], "ln3_b": g["ln3_b"],
        "cw0": conv_w[:, 0], "cw1": conv_w[:, 1], "cw2": conv_w[:, 2],
        "cb": g["conv_b"], "bo_eff": bo_eff, "bq": g["bqkv"][:D],
        "bk": g["bqkv"][D:2 * D], "b2": g["b2"],
    }
    vecs = np.zeros((128, 4 * len(VEC_NAMES)), f32)
    for i, nme in enumerate(VEC_NAMES):
        vecs[:, 4 * i:4 * i + 4] = vec_vals[nme].reshape(DT, 128).T
    shared["vecs"] = vecs

    per_core = []
    for c in range(NCORES):
        b, half = c // 2, c % 2
        t0 = half * TLOC
        xT = np.ascontiguousarray(x[b].T)                      # (512, 2048)
        xrot = np.roll(xT, -(t0 - 1), axis=1)                  # ext col i = token t0-1+i
        mask = np.ones((128, TEXT), bf)
        if half == 0:
            mask[:, 0] = 0.0
        else:
            mask[:, TEXT - 1] = 0.0
        im = dict(shared)
        im["xT"] = np.ascontiguousarray(xrot.reshape(DT, 128, S)).astype(f32)
        im["mask"] = mask
        per_core.append(im)
    return flags, per_core


_PROG_CACHE = {}


def get_program(flags, stage=6):
    key = (tuple(sorted(flags.items())), stage)
    if key not in _PROG_CACHE:
        _PROG_CACHE[key] = build_program(flags, stage)
    return _PROG_CACHE[key]


def run(inputs, **spmd_kwargs):
    """Run on hardware; returns (output (4,2048,512) f32, BassKernelResults)."""
    flags, per_core = prepare(inputs)
    nc = get_program(flags)
    res = run_bass_kernel_spmd(nc, per_core, core_ids=list(range(NCORES)),
                               **spmd_kwargs)
    out = np.empty((B, S, D), np.float32)
    for c in range(NCORES):
        b, half = c // 2, c % 2
        t0 = half * TLOC
        yT = res.results[c]["yT"].reshape(D, TLOC)
        out[b, t0:t0 + TLOC, :] = yT.T
    return out, res


def kernel(**inputs) -> np.ndarray:
    out, _ = run(inputs)
    return out


def _make_sharded(nc, reps_unused=None):
    import jax
    from jax.sharding import Mesh, PartitionSpec
    from jax.experimental.shard_map import shard_map
    from concourse import bass2jax as b2j
    import concourse.mybir as _mybir

    b2j.install_neuronx_cc_hook()
    fn0 = nc.m.functions[0]
    pid_name = nc.partition_id_tensor.name if nc.partition_id_tensor else None
    in_names, out_names, out_avals, zero_outs = [], [], [], []
    for alloc in fn0.allocations:
        if not isinstance(alloc, _mybir.MemoryLocationSet):
            continue
        name = alloc.memorylocations[0].name
        if alloc.kind == "ExternalInput":
            if name != pid_name:
                in_names.append(name)
        elif alloc.kind == "ExternalOutput":
            out_names.append(name)
            shape = tuple(alloc.tensor_shape)
            dt = _mybir.dt.np(alloc.dtype)
            out_avals.append(jax.core.ShapedArray(shape, dt))
            zero_outs.append(np.zeros(shape, dt))
    n_params = len(in_names)
    all_names = list(in_names) + list(out_names)
    if pid_name is not None:
        all_names.append(pid_name)

    def body(*args):
        operands = list(args)
        if pid_name is not None:
            operands.append(b2j.partition_id_tensor())
        outs = b2j._bass_exec_p.bind(
            *operands,
            out_avals=tuple(out_avals), in_names=tuple(all_names),
            out_names=tuple(out_names), lowering_input_output_aliases=(),
            sim_require_finite=True, sim_require_nnan=True, nc=nc)
        return tuple(outs)

    devices = jax.devices()[:NCORES]
    mesh = Mesh(np.asarray(devices), ("core",))
    P = PartitionSpec
    nin = n_params + len(out_names)
    sharded = jax.jit(shard_map(body, mesh=mesh, in_specs=(P("core"),) * nin,
                                out_specs=(P("core"),) * len(out_names),
                                check_rep=False))
    return sharded, in_names, zero_outs


def _time_dispatch(sharded, concat_in, iters):
    import time as _time
    import jax
    r = sharded(*concat_in)
    jax.block_until_ready(r)
    ts = []
    for _ in range(iters):
        t0 = _time.perf_counter()
        r = sharded(*concat_in)
        jax.block_until_ready(r)
        ts.append(_time.perf_counter() - t0)
    ts.sort()
    return ts[len(ts) // 4]  # lower quartile


def _baseline_nc():
    """Minimal program through the same path, to estimate dispatch overhead."""
    nc = bacc.Bacc("TRN2", target_bir_lowering=False, debug=False)
    xi = nc.dram_tensor("bx", (128, 128), F32, kind="ExternalInput").ap()
    yo = nc.dram_tensor("by", (128, 128), F32, kind="ExternalOutput").ap()
    with tile.TileContext(nc) as tc:
        with tc.tile_pool(name="sb", bufs=1) as sb:
            t = sb.tile((128, 128), F32, name="bt", tag="bt")
            nc.sync.dma_start(out=t, in_=xi)
            nc.sync.dma_start(out=yo, in_=t)
    nc.compile()
    return nc


def timed_run(inputs, reps=30, batches=3):
    """Estimate on-device exec time: single-dispatch wall time minus the
    dispatch overhead of a minimal kernel through the same path."""
    flags, per_core = prepare(inputs)
    nc = get_program(flags)
    sharded, in_names, zero_outs = _make_sharded(nc)
    concat_in = [np.concatenate([np.asarray(per_core[c][nm]) for c in range(NCORES)],
                                axis=0) for nm in in_names]
    concat_in += [np.concatenate([z] * NCORES, axis=0) for z in zero_outs]
    t_full = _time_dispatch(sharded, concat_in, reps)

    print(f"  dispatch(full)={t_full*1e6:.0f}us (upper bound incl. host dispatch)")
    return t_full * 1e9


def kernel(**inputs) -> np.ndarray:
    out, _ = run(inputs)
    return out


def timed_run(inputs, reps=30, batches=3):
    """Time repeated on-device executes of the compiled program (test helper).

    Replicates bass2jax.run_bass_via_pjrt's multi-core path, but keeps inputs
    device-resident and chains `reps` sequential executes inside one jit (a
    zero-valued scalar from each iteration's output is added to a small input
    of the next to prevent CSE/reordering). Returns best per-iteration ns.
    """
    import time as _time
    import jax
    from jax.sharding import Mesh, PartitionSpec
    from jax.experimental.shard_map import shard_map
    from concourse import bass2jax as b2j
    import concourse.mybir as _mybir

    flags, per_core = prepare(inputs)
    nc = get_program(flags)
    b2j.install_neuronx_cc_hook()

    fn0 = nc.m.functions[0]
    pid_name = nc.partition_id_tensor.name if nc.partition_id_tensor else None
    in_names, out_names, out_avals, zero_outs = [], [], [], []
    for alloc in fn0.allocations:
        if not isinstance(alloc, _mybir.MemoryLocationSet):
            continue
        name = alloc.memorylocations[0].name
        if alloc.kind == "ExternalInput":
            if name != pid_name:
                in_names.append(name)
        elif alloc.kind == "ExternalOutput":
            out_names.append(name)
            shape = tuple(alloc.tensor_shape)
            dt = _mybir.dt.np(alloc.dtype)
            out_avals.append(jax.core.ShapedArray(shape, dt))
            zero_outs.append(np.zeros(shape, dt))
    n_params = len(in_names)
    all_names = tuple(in_names + out_names)
    vidx = in_names.index("vecs")

    if pid_name is not None:
        all_names = tuple(list(all_names) + [pid_name])

    def body(*args):
        arrs = list(args[:n_params])
        zeros = list(args[n_params:])
        outs = None
        for _ in range(reps):
            operands = arrs + zeros
            if pid_name is not None:
                operands = operands + [b2j.partition_id_tensor()]
            outs = b2j._bass_exec_p.bind(
                *operands,
                out_avals=tuple(out_avals), in_names=all_names,
                out_names=tuple(out_names), lowering_input_output_aliases=(),
                sim_require_finite=True, sim_require_nnan=True, nc=nc)
            arrs[vidx] = arrs[vidx] + outs[0].reshape(-1)[0] * 0.0
        return tuple(outs)

    devices = jax.devices()[:NCORES]
    mesh = Mesh(np.asarray(devices), ("core",))
    P = PartitionSpec
    nin = n_params + len(out_names)
    sharded = jax.jit(shard_map(body, mesh=mesh, in_specs=(P("core"),) * nin,
                                out_specs=(P("core"),) * len(out_names),
                                check_rep=False))
    concat_in = [np.concatenate([np.asarray(per_core[c][nm]) for c in range(NCORES)], axis=0)
                 for nm in in_names]
    concat_in += [np.concatenate([z] * NCORES, axis=0) for z in zero_outs]
    r = sharded(*concat_in)
    jax.block_until_ready(r)
    best = float("inf")
    for _ in range(batches):
        t0 = _time.perf_counter()
        r = sharded(*concat_in)
        jax.block_until_ready(r)
        dt_s = _time.perf_counter() - t0
        best = min(best, dt_s / reps)
    return best * 1e9


# revision 34
# speedup vs baseline: 1.0365x; 1.0109x over previous
"""Trainium2 Bass kernel for an enhanced transformer block (attn + depthwise-conv + MLP).

Sharding: 8 cores = 4 batches x 2 sequence halves (data parallel, no collectives).
Each core receives its batch's x TRANSPOSED (feature-major: d on partitions,
tokens on the free axis) and ROTATED so that its extended token range
[t0-1, t1+1) lands at columns [0, 1026) uniformly on every core (SPMD: one
program, different data). K/V are computed over the full (rotated) sequence;
q/attention only over the core's 1026 extended columns. The rotation makes
attention sums run over a permuted key order, which is mathematically
identical. Halo columns provide the depthwise-conv neighbor values; at
sequence edges the halo is dead (wrapped garbage) and is zeroed via a mask
folded into LN2's rstd.

Softmax is computed without max-subtraction (scores for this problem are
O(1); exp cannot overflow) so the denominator can be accumulated by an
extra all-ones column appended to V in the P@V matmul.
"""

import numpy as np
import ml_dtypes

import concourse.bass as bass
import concourse.bacc as bacc
import concourse.mybir as mybir
import concourse.tile as tile
from concourse.bass_utils import run_bass_kernel_spmd

F32 = mybir.dt.float32
F32R = mybir.dt.float32r
BF16 = mybir.dt.bfloat16
Alu = mybir.AluOpType
Act = mybir.ActivationFunctionType

D = 512          # model dim
S = 2048         # sequence length
B = 4            # batch
H = 8            # heads
HD = 64          # head dim
DFF = 2048       # mlp hidden
NCORES = 8
TLOC = 1024      # local tokens per core
TEXT = 1026      # extended (1 halo col each side)
DT = 4           # d-tiles of 128
EPS = 1e-5

# order of packed 512-length vectors in the "vecs" input
VEC_NAMES = ["ln1_g", "ln1_b", "ln2_g", "ln2_b", "lnc_g", "lnc_b",
             "ln3_g", "ln3_b", "cw0", "cw1", "cw2", "cb",
             "bo_eff", "bq", "bk", "b2"]
VIDX = {n: i for i, n in enumerate(VEC_NAMES)}


def _vap(vecs_sb, name, dt):
    """per-partition [128,1] scalar AP for vector `name`, d-tile dt."""
    c = 4 * VIDX[name] + dt
    return vecs_sb[:, c:c + 1]


def build_program(flags, stage=6):
    """Trace the uniform per-core program. flags: dict of bools enabling
    optional bias/scale terms (specialized to the actual input values).
    stage<6 emits an intermediate tensor and stops (debug bisection)."""
    nc = bacc.Bacc("TRN2", target_bir_lowering=False, debug=False)

    xT_d = nc.dram_tensor("xT", (DT, 128, S), F32, kind="ExternalInput").ap()
    wqkvT_d = nc.dram_tensor("wqkvT", (DT, 128, 3 * D), BF16, kind="ExternalInput").ap()
    woT_d = nc.dram_tensor("woT", (DT, 128, D), BF16, kind="ExternalInput").ap()
    w1T_d = nc.dram_tensor("w1T", (DT, 128, DFF), BF16, kind="ExternalInput").ap()
    w2T_d = nc.dram_tensor("w2T", (16, 128, D), BF16, kind="ExternalInput").ap()
    vecs_d = nc.dram_tensor("vecs", (128, 4 * len(VEC_NAMES)), F32, kind="ExternalInput").ap()
    b1m_d = nc.dram_tensor("b1m", (128, 16), F32, kind="ExternalInput").ap()
    mask_d = nc.dram_tensor("mask", (128, TEXT), BF16, kind="ExternalInput").ap()
    yT_d = nc.dram_tensor("yT", (DT, 128, TLOC), F32, kind="ExternalOutput").ap()

    with tile.TileContext(nc) as tc:
        _prog(nc, tc, flags,
              xT_d, wqkvT_d, woT_d, w1T_d, w2T_d, vecs_d, b1m_d, mask_d, yT_d,
              stage=stage)
    nc.compile()
    return nc


def _ln_stats(nc, lnps, lnw, ones, eps_sb, z_tiles, sl, n):
    """LN stats over the d axis (partitions x 4 tiles) for token cols `sl`
    (length n). Returns (mu_rep, r_rep) fp32 SBUF tiles (128, n), replicated
    across partitions. z_tiles: 4 fp32 SBUF tiles (128, >=n cols)."""
    s1 = lnps.tile((128, 512), F32, name="s1", tag="s1", bufs=2)
    s2 = lnps.tile((128, 512), F32, name="s2", tag="s2", bufs=2)
    for dt in range(DT):
        xb = lnw.tile((128, 512), BF16, name="xb", tag="xb", bufs=4)
        nc.vector.tensor_copy(xb[:, :n], z_tiles[dt][:, sl])
        nc.tensor.matmul(s1[:, :n], lhsT=ones, rhs=xb[:, :n],
                         start=(dt == 0), stop=(dt == DT - 1))
        sq = lnw.tile((128, 512), BF16, name="sq", tag="sq", bufs=4)
        nc.scalar.square(sq[:, :n], z_tiles[dt][:, sl])
        nc.tensor.matmul(s2[:, :n], lhsT=ones, rhs=sq[:, :n],
                         start=(dt == 0), stop=(dt == DT - 1))
    mu = lnw.tile((128, 512), F32, name="mu", tag="mu")
    nc.vector.tensor_scalar_mul(out=mu[:, :n], in0=s1[:, :n], scalar1=1.0 / D)
    mu2 = lnw.tile((128, 512), F32, name="mu2", tag="scratch", bufs=3)
    nc.vector.tensor_mul(mu2[:, :n], mu[:, :n], mu[:, :n])
    m2s = lnw.tile((128, 512), F32, name="m2s", tag="scratch", bufs=3)
    nc.vector.tensor_scalar_mul(out=m2s[:, :n], in0=s2[:, :n], scalar1=1.0 / D)
    var = lnw.tile((128, 512), F32, name="var", tag="var")
    nc.vector.tensor_sub(var[:, :n], m2s[:, :n], mu2[:, :n])
    sd = lnw.tile((128, 512), F32, name="sd", tag="scratch", bufs=3)
    nc.scalar.activation(sd[:, :n], var[:, :n], Act.Sqrt, bias=eps_sb[:, 0:1])
    r = lnw.tile((128, 512), F32, name="r", tag="r")
    nc.vector.reciprocal(r[:, :n], sd[:, :n])
    return mu, r


def _ln_apply(nc, lnw, vecs_sb, z_tiles, out_tiles, sl, n, mu, r,
              gname, bname, gflag, bflag, out_sl=None):
    """out = (z - mu) * r [* g] [+ b] for each d-tile, cols sl."""
    osl = sl if out_sl is None else out_sl
    for dt in range(DT):
        xc = lnw.tile((128, 512), F32, name="xc", tag="xc", bufs=2)
        nc.vector.tensor_sub(xc[:, :n], z_tiles[dt][:, sl], mu[:, :n])
        dst = out_tiles[dt][:, osl]
        if gflag:
            nc.vector.scalar_tensor_tensor(out=dst, in0=xc[:, :n],
                                           scalar=_vap(vecs_sb, gname, dt),
                                           in1=r[:, :n], op0=Alu.mult, op1=Alu.mult)
        else:
            nc.vector.tensor_mul(dst, xc[:, :n], r[:, :n])
        if bflag:
            nc.vector.tensor_scalar_add(out=dst, in0=dst,
                                        scalar1=_vap(vecs_sb, bname, dt))


def _prog(nc, tc, fl, xT_d, wqkvT_d, woT_d, w1T_d, w2T_d, vecs_d, b1m_d,
          mask_d, yT_d, stage=6):
    Ls, Rs, Ps = [], [], []  # open-pool stacks (left / right / psum)

    def _dbg_exit(tiles):
        dbg = tc.alloc_tile_pool(name="dbgout", bufs=1)
        for dt in range(DT):
            t = dbg.tile((128, TLOC), F32, name=f"dbg{dt}", tag=f"dbg{dt}")
            nc.vector.tensor_copy(t, tiles[dt][:, 0:TLOC])
            nc.sync.dma_start(out=yT_d[dt], in_=t)
        dbg.release()
        for st in (Ps, Ls, Rs):
            while st:
                st.pop().release()

    # ---------------- persistent pools ----------------
    consts = tc.alloc_tile_pool(name="consts", bufs=1); Ls.append(consts)
    wts = tc.alloc_tile_pool(name="wts", bufs=1); Ls.append(wts)
    lnw = tc.alloc_tile_pool(name="lnw", bufs=2); Ls.append(lnw)
    small = tc.alloc_tile_pool(name="small", bufs=2); Ls.append(small)

    vecs_sb = consts.tile((128, 4 * len(VEC_NAMES)), F32, name="vecs_sb", tag="vecs")
    nc.sync.dma_start(out=vecs_sb, in_=vecs_d)
    b1_sb = consts.tile((128, 16), F32, name="b1_sb", tag="b1")
    nc.sync.dma_start(out=b1_sb, in_=b1m_d)
    mask_sb = consts.tile((128, TEXT), BF16, name="mask_sb", tag="mask")
    nc.sync.dma_start(out=mask_sb, in_=mask_d)
    ones = consts.tile((128, 128), BF16, name="ones", tag="ones")
    nc.vector.memset(ones, 1.0)
    eps_sb = consts.tile((128, 1), F32, name="eps_sb", tag="eps")
    nc.vector.memset(eps_sb, EPS)

    wqkv_sb = []
    for dt in range(DT):
        t = wts.tile((128, 3 * D), BF16, name=f"wqkv{dt}", tag=f"wqkv{dt}")
        nc.sync.dma_start(out=t, in_=wqkvT_d[dt])
        wqkv_sb.append(t)
    wo_sb = []
    for dt in range(DT):
        t = wts.tile((128, D), BF16, name=f"wo{dt}", tag=f"wo{dt}")
        nc.sync.dma_start(out=t, in_=woT_d[dt])
        wo_sb.append(t)

    # x_res: residual slice of x (cols 0:TEXT), outlives the full-x tiles
    xres_pool = tc.alloc_tile_pool(name="xres_pool", bufs=1, side="right"); Rs.append(xres_pool)
    xres_sb = [xres_pool.tile((128, TEXT), F32, name=f"xr{dt}", tag=f"xr{dt}")
               for dt in range(DT)]
    # aT (attention output, feature-major) - lives until out-proj
    a_pool = tc.alloc_tile_pool(name="a_pool", bufs=1, side="right"); Rs.append(a_pool)
    a_sb = [a_pool.tile((128, TEXT), BF16, name=f"a{dt}", tag=f"a{dt}")
            for dt in range(DT)]
    # k/v/q - live until end of attention
    kvq = tc.alloc_tile_pool(name="kvq", bufs=1, side="right"); Rs.append(kvq)

    # hT (LN1 output, bf16) - lives until end of QKV
    h_pool = tc.alloc_tile_pool(name="h_pool", bufs=1); Ls.append(h_pool)
    h_sb = [h_pool.tile((128, S), BF16, name=f"h{dt}", tag=f"h{dt}")
            for dt in range(DT)]

    # x tiles (feature-major, rotated), full sequence
    x_pool = tc.alloc_tile_pool(name="x_pool", bufs=1); Ls.append(x_pool)
    x_sb = []
    for dt in range(DT):
        t = x_pool.tile((128, S), F32, name=f"x{dt}", tag=f"x{dt}")
        nc.sync.dma_start(out=t, in_=xT_d[dt])
        x_sb.append(t)

    # ---------------- phase 1: LN1 over full sequence -> hT (bf16) --------
    ln1ps = tc.alloc_tile_pool(name="ln1ps", bufs=2, space="PSUM"); Ps.append(ln1ps)
    with nc.named_scope("ln1"):
        for ch in range(4):
            sl = slice(ch * 512, ch * 512 + 512)
            mu, r = _ln_stats(nc, ln1ps, lnw, ones, eps_sb, x_sb, sl, 512)
            _ln_apply(nc, lnw, vecs_sb, x_sb, h_sb, sl, 512, mu, r,
                      "ln1_g", "ln1_b", fl["ln1_g"], fl["ln1_b"])
    Ps.pop().release()
    for dt in range(DT):
        nc.vector.tensor_copy(xres_sb[dt], x_sb[dt][:, 0:TEXT])
    Ls.pop().release()  # x_pool
    if stage == 1:
        return _dbg_exit(h_sb)

    # ---------------- phase 2: QKV ----------------
    k_sb = [kvq.tile((128, S), BF16, name=f"k{dt}", tag=f"k{dt}") for dt in range(DT)]
    v_sb = [kvq.tile((128, H, HD + 1), BF16, name=f"v{tc_}", tag=f"v{tc_}")
            for tc_ in range(16)]
    q_sb = [kvq.tile((128, TEXT), BF16, name=f"q{dt}", tag=f"q{dt}")
            for dt in range(DT)]

    qkvps = tc.alloc_tile_pool(name="qkvps", bufs=4, space="PSUM"); Ps.append(qkvps)
    with nc.named_scope("qkv"):
        # k: feature-major (j on partitions, tokens free)
        for jt in range(DT):
            for ch in range(4):
                sl = slice(ch * 512, ch * 512 + 512)
                ps = qkvps.tile((128, 512), F32, name="kps", tag="mm")
                for dt in range(DT):
                    nc.tensor.matmul(ps, lhsT=wqkv_sb[dt][:, D + jt * 128: D + jt * 128 + 128],
                                     rhs=h_sb[dt][:, sl],
                                     start=(dt == 0), stop=(dt == DT - 1))
                if fl["bk"]:
                    nc.scalar.add(out=k_sb[jt][:, sl], in_=ps,
                                  add=_vap(vecs_sb, "bk", jt))
                else:
                    nc.scalar.copy(k_sb[jt][:, sl], ps)
        # q: feature-major, extended token range only
        for jt in range(DT):
            for (c0, n) in ((0, 512), (512, 512), (1024, 2)):
                tag = "mm" if n == 512 else "qtiny"
                ps = qkvps.tile((128, 512) if n == 512 else (128, 2), F32,
                                name="qps", tag=tag, bufs=4 if n == 512 else 2)
                for dt in range(DT):
                    nc.tensor.matmul(ps[:, :n], lhsT=wqkv_sb[dt][:, jt * 128: jt * 128 + 128],
                                     rhs=h_sb[dt][:, c0:c0 + n],
                                     start=(dt == 0), stop=(dt == DT - 1))
                if fl["bq"]:
                    nc.scalar.add(out=q_sb[jt][:, c0:c0 + n], in_=ps[:, :n],
                                  add=_vap(vecs_sb, "bq", jt))
                else:
                    nc.scalar.copy(q_sb[jt][:, c0:c0 + n], ps[:, :n])
        # v: token-major (tokens on partitions, j free), with ones column
        for tc_ in range(16):
            nc.vector.memset(v_sb[tc_][:, :, HD:HD + 1], 1.0)
            ps = qkvps.tile((128, 512), F32, name="vps", tag="mm")
            for dt in range(DT):
                nc.tensor.matmul(ps, lhsT=h_sb[dt][:, tc_ * 128: tc_ * 128 + 128],
                                 rhs=wqkv_sb[dt][:, 2 * D:3 * D],
                                 start=(dt == 0), stop=(dt == DT - 1))
            src = ps[:, :].rearrange("p (h d) -> p h d", h=H)
            # v bias would be per-free here; it is folded into bo_eff on host.
            nc.scalar.copy(v_sb[tc_][:, :, 0:HD], src)
    Ps.pop().release()  # qkvps
    Ls.pop().release()  # h_pool
    if stage == 2:
        return _dbg_exit(k_sb)

    # ---------------- phase 3: attention ----------------
    p_pool = tc.alloc_tile_pool(name="p_pool", bufs=6, side="right"); Rs.append(p_pool)
    scps = tc.alloc_tile_pool(name="scps", bufs=4, space="PSUM"); Ps.append(scps)
    avps = tc.alloc_tile_pool(name="avps", bufs=2, space="PSUM"); Ps.append(avps)

    with nc.named_scope("attn"):
        for hp in range(4):  # head pairs: a=2hp (rows 0:64), b=2hp+1 (rows 64:128)
            av_ab = [avps.tile((128, 1024), F32, name=f"av{hp}_{i}", tag="av")
                     for i in range(2)]
            rows = [slice(0, 64), slice(64, 128)]
            for kc in range(16):
                ksl = slice(kc * 128, kc * 128 + 128)
                ptiles = [None, None]
                for i in range(2):
                    sc = scps.tile((128, 1024), F32, name="sc", tag="sc", bufs=2)
                    for qc in range(2):
                        nc.tensor.matmul(sc[:, qc * 512:(qc + 1) * 512],
                                         lhsT=k_sb[hp][rows[i], ksl],
                                         rhs=q_sb[hp][rows[i], qc * 512:(qc + 1) * 512],
                                         start=True, stop=True)
                    pt = p_pool.tile((128, 1024), BF16, name="pt", tag="pt")
                    nc.scalar.activation(pt, sc, Act.Exp, scale=0.125)
                    ptiles[i] = pt
                # av accumulation
                for i in range(2):
                    for qc in range(2):
                        nc.tensor.matmul(av_ab[i][0:HD + 1, qc * 512:(qc + 1) * 512],
                                         lhsT=v_sb[kc][:, 2 * hp + i, :],
                                         rhs=ptiles[i][:, qc * 512:(qc + 1) * 512],
                                         start=(kc == 0), stop=(kc == 15))
            # normalize: recip of denominator row, replicate via K=1 matmul,
            # stage to SBUF (DVE reads only one PSUM operand), multiply
            for i in range(2):
                if stage == 31:
                    nc.vector.tensor_copy(a_sb[hp][rows[i], 0:1024],
                                          av_ab[i][0:64, :])
                    continue
                rec = small.tile((1, 1024), BF16, name="rec", tag="rec")
                with nc.allow_low_precision("bf16 softmax denom recip (attn out is tiny)"):
                    nc.vector.reciprocal(rec, av_ab[i][HD:HD + 1, :])
                for qc in range(2):
                    qsl = slice(qc * 512, qc * 512 + 512)
                    nc.tensor.matmul(av_ab[i][64:128, qsl],
                                     lhsT=ones[0:1, 0:64], rhs=rec[:, qsl],
                                     start=True, stop=True)
                rrep = small.tile((64, 1024), BF16, name="rrep", tag="rrep")
                nc.vector.tensor_copy(rrep, av_ab[i][64:128, :])
                nc.vector.tensor_tensor(a_sb[hp][rows[i], 0:1024],
                                        av_ab[i][0:64, :], rrep,
                                        Alu.mult)
    Ps.pop().release(); Ps.pop().release()  # avps scps
    Rs.pop().release()  # p_pool
    if stage in (3, 31, 32):
        Rs.pop().release()  # kvq
        return _dbg_exit(a_sb)

    # ---------------- phase 4: out-proj + residual -> x1 ----------------
    x2p = tc.alloc_tile_pool(name="x2p", bufs=1); Ls.append(x2p)
    x2_sb = [x2p.tile((128, TLOC), F32, name=f"x2_{dt}", tag=f"x2_{dt}")
             for dt in range(DT)]
    mid = tc.alloc_tile_pool(name="mid", bufs=1); Ls.append(mid)
    x1_sb = [mid.tile((128, TEXT), F32, name=f"x1_{dt}", tag=f"x1_{dt}")
             for dt in range(DT)]
    ops = tc.alloc_tile_pool(name="ops", bufs=4, space="PSUM"); Ps.append(ops)
    QC3 = ((0, 342), (342, 342), (684, 342))
    # -- halo attention (2 ext cols per core), token-major scores --
    phd_d = nc.dram_tensor("phd_scratch", (H, 2, S), BF16).ap()
    dsum_d = nc.dram_tensor("dsum_scratch", (H, 2, 1), F32).ap()
    hps = tc.alloc_tile_pool(name="hps", bufs=1, space="PSUM"); Ps.append(hps)
    hsb = tc.alloc_tile_pool(name="hsb", bufs=2)
    with nc.named_scope("halo"):
        for h in range(H):
            hp, i = h // 2, h % 2
            rws = slice(64 * i, 64 * i + 64)
            ph = hsb.tile((2, S), BF16, name="ph", tag="ph", bufs=1)
            dsum = hsb.tile((2, 2), F32, name="dsum", tag="dsum")
            for c2 in range(2):
                sch = hps.tile((2, 1024), F32, name="sch", tag="sch", bufs=1)
                for c in range(2):
                    cc = 2 * c2 + c
                    nc.tensor.matmul(sch[:, c * 512:(c + 1) * 512],
                                     lhsT=q_sb[hp][rws, 1024:1026],
                                     rhs=k_sb[hp][rws, cc * 512:(cc + 1) * 512],
                                     start=True, stop=True)
                nc.scalar.activation(ph[:, c2 * 1024:(c2 + 1) * 1024], sch,
                                     Act.Exp, scale=0.125,
                                     accum_out=dsum[:, c2:c2 + 1])
            nc.vector.tensor_add(dsum[:, 0:1], dsum[:, 0:1], dsum[:, 1:2])
            nc.sync.dma_start(out=phd_d[h], in_=ph)
            nc.sync.dma_start(out=dsum_d[h], in_=dsum[:, 0:1])
            pT = hsb.tile((128, 16, 2), BF16, name="pT", tag="pT")
            for q in range(2):
                nc.sync.dma_start(out=pT[:, :, q],
                                  in_=phd_d[h][q].rearrange("(c p) -> p c", p=128))
            denT = hsb.tile((1, 2), F32, name="denT", tag="denT")
            nc.sync.dma_start(out=denT, in_=dsum_d[h].rearrange("q one -> one q"))
            avh = hps.tile((128, 2), F32, name="avh", tag="avh", bufs=2)
            for kc in range(16):
                nc.tensor.matmul(avh[0:64, :], lhsT=v_sb[kc][:, h, 0:HD],
                                 rhs=pT[:, kc, :], start=(kc == 0), stop=(kc == 15))
            rec2 = hsb.tile((1, 2), BF16, name="rec2", tag="rec2")
            with nc.allow_low_precision("bf16 halo softmax recip"):
                nc.vector.reciprocal(rec2, denT)
            nc.tensor.matmul(avh[64:128, :], lhsT=ones[0:1, 0:64], rhs=rec2,
                             start=True, stop=True)
            rr2 = hsb.tile((64, 2), BF16, name="rr2", tag="rr2")
            nc.vector.tensor_copy(rr2, avh[64:128, :])
            nc.vector.tensor_tensor(a_sb[hp][rws, 1024:1026], avh[0:64, :],
                                    rr2, Alu.mult)
    hsb.release()
    Ps.pop().release()  # hps
    Rs.pop().release()  # kvq
    with nc.named_scope("outproj"):
        for jt in range(DT):
            for (c0, n) in QC3:
                sl = slice(c0, c0 + n)
                ps = ops.tile((128, 342), F32, name="ops_t", tag="o")
                for dt in range(DT):
                    nc.tensor.matmul(ps[:, :n], lhsT=wo_sb[dt][:, jt * 128: jt * 128 + 128],
                                     rhs=a_sb[dt][:, sl],
                                     start=(dt == 0), stop=(dt == DT - 1))
                if fl["bo"]:
                    nc.vector.scalar_tensor_tensor(out=x1_sb[jt][:, sl], in0=ps[:, :n],
                                                   scalar=_vap(vecs_sb, "bo_eff", jt),
                                                   in1=xres_sb[jt][:, sl],
                                                   op0=Alu.add, op1=Alu.add)
                else:
                    nc.vector.tensor_tensor(x1_sb[jt][:, sl], ps[:, :n],
                                            xres_sb[jt][:, sl], Alu.add)
    Ps.pop().release()  # ops
    Rs.pop().release()  # a_pool
    Rs.pop().release()  # xres_pool
    if stage == 4:
        return _dbg_exit(x1_sb)

    # ---------------- phase 5: conv block -> x2 ----------------
    h2_sb = [mid.tile((128, TEXT), F32, name=f"h2_{dt}", tag=f"h2_{dt}")
             for dt in range(DT)]
    conv_t = tc.alloc_tile_pool(name="conv_t", bufs=1); Ls.append(conv_t)
    tcv = [conv_t.tile((128, TLOC), F32, name=f"tc{dt}", tag=f"tc{dt}")
           for dt in range(DT)]
    g_sb = [conv_t.tile((128, TLOC), F32, name=f"g{dt}", tag=f"g{dt}")
            for dt in range(DT)]

    cps = tc.alloc_tile_pool(name="cps", bufs=2, space="PSUM"); Ps.append(cps)
    with nc.named_scope("convblock"):
        # LN2 over 1026 cols (3 chunks of 342), rstd masked at dead halo cols
        for (c0, n) in QC3:
            sl = slice(c0, c0 + n)
            mu, r = _ln_stats(nc, cps, lnw, ones, eps_sb, x1_sb, sl, n)
            nc.vector.tensor_mul(r[:, :n], r[:, :n], mask_sb[:, sl])
            _ln_apply(nc, lnw, vecs_sb, x1_sb, h2_sb, sl, n, mu, r,
                      "ln2_g", "ln2_b", fl["ln2_g"], fl["ln2_b"])
        # depthwise conv along tokens (output = local cols [1,1025) -> 1024)
        for dt in range(DT):
            tmp = conv_t.tile((128, TLOC), F32, name="ctmp", tag="ctmp", bufs=2)
            if fl["cb"]:
                nc.vector.tensor_scalar(out=tmp, in0=h2_sb[dt][:, 0:TLOC],
                                        scalar1=_vap(vecs_sb, "cw0", dt),
                                        scalar2=_vap(vecs_sb, "cb", dt),
                                        op0=Alu.mult, op1=Alu.add)
            else:
                nc.vector.tensor_scalar_mul(out=tmp, in0=h2_sb[dt][:, 0:TLOC],
                                            scalar1=_vap(vecs_sb, "cw0", dt))
            nc.vector.scalar_tensor_tensor(out=tmp, in0=h2_sb[dt][:, 1:TLOC + 1],
                                           scalar=_vap(vecs_sb, "cw1", dt),
                                           in1=tmp, op0=Alu.mult, op1=Alu.add)
            nc.vector.scalar_tensor_tensor(out=tcv[dt], in0=h2_sb[dt][:, 2:TLOC + 2],
                                           scalar=_vap(vecs_sb, "cw2", dt),
                                           in1=tmp, op0=Alu.mult, op1=Alu.add)
        # LNc on conv output (local 1024), then gelu
        for ch in range(2):
            sl = slice(ch * 512, ch * 512 + 512)
            mu, r = _ln_stats(nc, cps, lnw, ones, eps_sb, tcv, sl, 512)
            _ln_apply(nc, lnw, vecs_sb, tcv, tcv, sl, 512, mu, r,
                      "lnc_g", "lnc_b", fl["lnc_g"], fl["lnc_b"])
        for dt in range(DT):
            nc.scalar.activation(g_sb[dt], tcv[dt], Act.Gelu)
        # x2 = x1 + h2 + gelu(...)  (local cols)
        for dt in range(DT):
            nc.vector.tensor_add(x2_sb[dt], x1_sb[dt][:, 1:TLOC + 1],
                                 h2_sb[dt][:, 1:TLOC + 1])
            nc.vector.tensor_add(x2_sb[dt], x2_sb[dt], g_sb[dt])
    Ps.pop().release()  # cps
    Ls.pop().release()  # conv_t
    Ls.pop().release()  # mid
    if stage == 5:
        return _dbg_exit(x2_sb)

    # ---------------- phase 6: MLP -> output ----------------
    mlpp = tc.alloc_tile_pool(name="mlpp", bufs=1); Ls.append(mlpp)
    h3_sb = [mlpp.tile((128, TLOC), BF16, name=f"h3_{dt}", tag=f"h3_{dt}")
             for dt in range(DT)]
    u_sb = [mlpp.tile((128, TLOC), BF16, name=f"u{jt}", tag=f"u{jt}")
            for jt in range(16)]
    out_sb = [mlpp.tile((128, TLOC), F32, name=f"o{dt}", tag=f"o{dt}")
              for dt in range(DT)]

    w1_sb = []
    for dt in range(DT):
        t = wts.tile((128, DFF), BF16, name=f"w1_{dt}", tag=f"w1_{dt}")
        nc.sync.dma_start(out=t, in_=w1T_d[dt])
        w1_sb.append(t)
    w2_sb = []
    for d2 in range(16):
        t = wts.tile((128, D), BF16, name=f"w2_{d2}", tag=f"w2_{d2}")
        nc.sync.dma_start(out=t, in_=w2T_d[d2])
        w2_sb.append(t)

    lps = tc.alloc_tile_pool(name="lps", bufs=2, space="PSUM"); Ps.append(lps)
    mps = tc.alloc_tile_pool(name="mps", bufs=2, space="PSUM"); Ps.append(mps)
    with nc.named_scope("mlp"):
        for ch in range(2):
            sl = slice(ch * 512, ch * 512 + 512)
            mu, r = _ln_stats(nc, lps, lnw, ones, eps_sb, x2_sb, sl, 512)
            _ln_apply(nc, lnw, vecs_sb, x2_sb, h3_sb, sl, 512, mu, r,
                      "ln3_g", "ln3_b", fl["ln3_g"], fl["ln3_b"])
        for jt in range(16):
            for ch in range(2):
                sl = slice(ch * 512, ch * 512 + 512)
                ps = lps.tile((128, 512), F32, name="ups", tag="ups", bufs=2)
                for dt in range(DT):
                    nc.tensor.matmul(ps, lhsT=w1_sb[dt][:, jt * 128: jt * 128 + 128],
                                     rhs=h3_sb[dt][:, sl],
                                     start=(dt == 0), stop=(dt == DT - 1))
                if fl["b1"]:
                    nc.scalar.activation(u_sb[jt][:, sl], ps, Act.Gelu,
                                         bias=b1_sb[:, jt:jt + 1])
                else:
                    nc.scalar.activation(u_sb[jt][:, sl], ps, Act.Gelu)
        for jt in range(DT):
            for ch in range(2):
                sl = slice(ch * 512, ch * 512 + 512)
                ps = mps.tile((128, 512), F32, name="mmps", tag="m")
                for d2 in range(16):
                    nc.tensor.matmul(ps, lhsT=w2_sb[d2][:, jt * 128: jt * 128 + 128],
                                     rhs=u_sb[d2][:, sl],
                                     start=(d2 == 0), stop=(d2 == 15))
                if fl["b2"]:
                    nc.vector.scalar_tensor_tensor(out=out_sb[jt][:, sl], in0=ps,
                                                   scalar=_vap(vecs_sb, "b2", jt),
                                                   in1=x2_sb[jt][:, sl],
                                                   op0=Alu.add, op1=Alu.add)
                else:
                    nc.vector.tensor_tensor(out_sb[jt][:, sl], ps,
                                            x2_sb[jt][:, sl], Alu.add)
            nc.sync.dma_start(out=yT_d[jt], in_=out_sb[jt])
    Ps.pop().release(); Ps.pop().release()  # mps lps
    Ls.pop().release()  # mlpp
    Ls.pop().release()  # x2p
    Ls.pop().release(); Ls.pop().release(); Ls.pop().release(); Ls.pop().release()
    x1_sb, h2_sb  # keep references


# ======================= host side =======================

def _nz(a):
    return bool(np.any(np.asarray(a) != 0))


def prepare(inputs):
    """Returns (flags, shared_inputs, per_core_inputs[8])."""
    f32 = np.float32
    g = {k: np.asarray(v, f32) for k, v in inputs.items()}
    x = g["x"]
    Wqkv, Wo, W1, W2 = g["Wqkv"], g["Wo"], g["W1"], g["W2"]
    conv_w = g["conv_w"]

    flags = {
        "ln1_g": not np.allclose(g["ln1_g"], 1.0), "ln1_b": _nz(g["ln1_b"]),
        "ln2_g": not np.allclose(g["ln2_g"], 1.0), "ln2_b": _nz(g["ln2_b"]),
        "lnc_g": not np.allclose(g["lnc_g"], 1.0), "lnc_b": _nz(g["lnc_b"]),
        "ln3_g": not np.allclose(g["ln3_g"], 1.0), "ln3_b": _nz(g["ln3_b"]),
        "bq": _nz(g["bqkv"][:D]), "bk": _nz(g["bqkv"][D:2 * D]),
        "cb": _nz(g["conv_b"]),
        "b1": _nz(g["b1"]), "b2": _nz(g["b2"]),
    }
    bv = g["bqkv"][2 * D:]
    bo_eff = g["bo"] + Wo @ bv
    flags["bo"] = _nz(bo_eff)

    bf = ml_dtypes.bfloat16
    shared = {
        "wqkvT": np.ascontiguousarray(Wqkv.T.reshape(DT, 128, 3 * D)).astype(bf),
        "woT": np.ascontiguousarray(Wo.T.reshape(DT, 128, D)).astype(bf),
        "w1T": np.ascontiguousarray(W1.T.reshape(DT, 128, DFF)).astype(bf),
        "w2T": np.ascontiguousarray(W2.T.reshape(16, 128, D)).astype(bf),
        "b1m": np.ascontiguousarray(g["b1"].reshape(16, 128).T).astype(f32),
    }
    vec_vals = {
        "ln1_g": g["ln1_g"], "ln1_b": g["ln1_b"], "ln2_g": g["ln2_g"],
        "ln2_b": g["ln2_b"], "lnc_g": g["lnc_g"], "lnc_b": g["lnc_b"],
        "ln3_g": g["ln3_g"], "ln3_b": g["ln3_b"],
        "cw0": conv_w[:, 0], "cw1": conv_w[:, 1], "cw2": conv_w[:, 2],
        "cb": g["conv_b"], "bo_eff": bo_eff, "bq": g["bqkv"][:D],
        "bk": g["bqkv"][D:2 * D], "b2": g["b2"],
    }
    vecs = np.zeros((128, 4 * len(VEC_NAMES)), f32)
    for i, nme in enumerate(VEC_NAMES):
        vecs[:, 4 * i:4 * i + 4] = vec_vals[nme].reshape(DT, 128).T
    shared["vecs"] = vecs

    per_core = []
    for c in range(NCORES):
        b, half = c // 2, c % 2
        t0 = half * TLOC
        xT = np.ascontiguousarray(x[b].T)                      # (512, 2048)
        xrot = np.roll(xT, -(t0 - 1), axis=1)                  # ext col i = token t0-1+i
        mask = np.ones((128, TEXT), bf)
        if half == 0:
            mask[:, 0] = 0.0
        else:
            mask[:, TEXT - 1] = 0.0
        im = dict(shared)
        im["xT"] = np.ascontiguousarray(xrot.reshape(DT, 128, S)).astype(f32)
        im["mask"] = mask
        per_core.append(im)
    return flags, per_core


_PROG_CACHE = {}


def get_program(flags, stage=6):
    key = (tuple(sorted(flags.items())), stage)
    if key not in _PROG_CACHE:
        _PROG_CACHE[key] = build_program(flags, stage)
    return _PROG_CACHE[key]


def run(inputs, **spmd_kwargs):
    """Run on hardware; returns (output (4,2048,512) f32, BassKernelResults)."""
    flags, per_core = prepare(inputs)
    nc = get_program(flags)
    res = run_bass_kernel_spmd(nc, per_core, core_ids=list(range(NCORES)),
                               **spmd_kwargs)
    out = np.empty((B, S, D), np.float32)
    for c in range(NCORES):
        b, half = c // 2, c % 2
        t0 = half * TLOC
        yT = res.results[c]["yT"].reshape(D, TLOC)
        out[b, t0:t0 + TLOC, :] = yT.T
    return out, res


def kernel(**inputs) -> np.ndarray:
    out, _ = run(inputs)
    return out


def _make_sharded(nc, reps_unused=None):
    import jax
    from jax.sharding import Mesh, PartitionSpec
    from jax.experimental.shard_map import shard_map
    from concourse import bass2jax as b2j
    import concourse.mybir as _mybir

    b2j.install_neuronx_cc_hook()
    fn0 = nc.m.functions[0]
    pid_name = nc.partition_id_tensor.name if nc.partition_id_tensor else None
    in_names, out_names, out_avals, zero_outs = [], [], [], []
    for alloc in fn0.allocations:
        if not isinstance(alloc, _mybir.MemoryLocationSet):
            continue
        name = alloc.memorylocations[0].name
        if alloc.kind == "ExternalInput":
            if name != pid_name:
                in_names.append(name)
        elif alloc.kind == "ExternalOutput":
            out_names.append(name)
            shape = tuple(alloc.tensor_shape)
            dt = _mybir.dt.np(alloc.dtype)
            out_avals.append(jax.core.ShapedArray(shape, dt))
            zero_outs.append(np.zeros(shape, dt))
    n_params = len(in_names)
    all_names = list(in_names) + list(out_names)
    if pid_name is not None:
        all_names.append(pid_name)

    def body(*args):
        operands = list(args)
        if pid_name is not None:
            operands.append(b2j.partition_id_tensor())
        outs = b2j._bass_exec_p.bind(
            *operands,
            out_avals=tuple(out_avals), in_names=tuple(all_names),
            out_names=tuple(out_names), lowering_input_output_aliases=(),
            sim_require_finite=True, sim_require_nnan=True, nc=nc)
        return tuple(outs)

    devices = jax.devices()[:NCORES]
    mesh = Mesh(np.asarray(devices), ("core",))
    P = PartitionSpec
    nin = n_params + len(out_names)
    sharded = jax.jit(shard_map(body, mesh=mesh, in_specs=(P("core"),) * nin,
                                out_specs=(P("core"),) * len(out_names),
                                check_rep=False))
    return sharded, in_names, zero_outs


def _time_dispatch(sharded, concat_in, iters):
    import time as _time
    import jax
    r = sharded(*concat_in)
    jax.block_until_ready(r)
    ts = []
    for _ in range(iters):
        t0 = _time.perf_counter()
        r = sharded(*concat_in)
        jax.block_until_ready(r)
        ts.append(_time.perf_counter() - t0)
    ts.sort()
    return ts[len(ts) // 4]  # lower quartile


def _baseline_nc():
    """Minimal program through the same path, to estimate dispatch overhead."""
    nc = bacc.Bacc("TRN2", target_bir_lowering=False, debug=False)
    xi = nc.dram_tensor("bx", (128, 128), F32, kind="ExternalInput").ap()
    yo = nc.dram_tensor("by", (128, 128), F32, kind="ExternalOutput").ap()
    with tile.TileContext(nc) as tc:
        with tc.tile_pool(name="sb", bufs=1) as sb:
            t = sb.tile((128, 128), F32, name="bt", tag="bt")
            nc.sync.dma_start(out=t, in_=xi)
            nc.sync.dma_start(out=yo, in_=t)
    nc.compile()
    return nc


def timed_run(inputs, reps=30, batches=3):
    """Estimate on-device exec time: single-dispatch wall time minus the
    dispatch overhead of a minimal kernel through the same path."""
    flags, per_core = prepare(inputs)
    nc = get_program(flags)
    sharded, in_names, zero_outs = _make_sharded(nc)
    concat_in = [np.concatenate([np.asarray(per_core[c][nm]) for c in range(NCORES)],
                                axis=0) for nm in in_names]
    concat_in += [np.concatenate([z] * NCORES, axis=0) for z in zero_outs]
    t_full = _time_dispatch(sharded, concat_in, reps)

    print(f"  dispatch(full)={t_full*1e6:.0f}us (upper bound incl. host dispatch)")
    return t_full * 1e9


def kernel(**inputs) -> np.ndarray:
    out, _ = run(inputs)
    return out


def timed_run(inputs, reps=30, batches=3):
    """Time repeated on-device executes of the compiled program (test helper).

    Replicates bass2jax.run_bass_via_pjrt's multi-core path, but keeps inputs
    device-resident and chains `reps` sequential executes inside one jit (a
    zero-valued scalar from each iteration's output is added to a small input
    of the next to prevent CSE/reordering). Returns best per-iteration ns.
    """
    import time as _time
    import jax
    from jax.sharding import Mesh, PartitionSpec
    from jax.experimental.shard_map import shard_map
    from concourse import bass2jax as b2j
    import concourse.mybir as _mybir

    flags, per_core = prepare(inputs)
    nc = get_program(flags)
    b2j.install_neuronx_cc_hook()

    fn0 = nc.m.functions[0]
    pid_name = nc.partition_id_tensor.name if nc.partition_id_tensor else None
    in_names, out_names, out_avals, zero_outs = [], [], [], []
    for alloc in fn0.allocations:
        if not isinstance(alloc, _mybir.MemoryLocationSet):
            continue
        name = alloc.memorylocations[0].name
        if alloc.kind == "ExternalInput":
            if name != pid_name:
                in_names.append(name)
        elif alloc.kind == "ExternalOutput":
            out_names.append(name)
            shape = tuple(alloc.tensor_shape)
            dt = _mybir.dt.np(alloc.dtype)
            out_avals.append(jax.core.ShapedArray(shape, dt))
            zero_outs.append(np.zeros(shape, dt))
    n_params = len(in_names)
    all_names = tuple(in_names + out_names)
    vidx = in_names.index("vecs")

    if pid_name is not None:
        all_names = tuple(list(all_names) + [pid_name])

    def body(*args):
        arrs = list(args[:n_params])
        zeros = list(args[n_params:])
        outs = None
        for _ in range(reps):
            operands = arrs + zeros
            if pid_name is not None:
                operands = operands + [b2j.partition_id_tensor()]
            outs = b2j._bass_exec_p.bind(
                *operands,
                out_avals=tuple(out_avals), in_names=all_names,
                out_names=tuple(out_names), lowering_input_output_aliases=(),
                sim_require_finite=True, sim_require_nnan=True, nc=nc)
            arrs[vidx] = arrs[vidx] + outs[0].reshape(-1)[0] * 0.0
        return tuple(outs)

    devices = jax.devices()[:NCORES]
    mesh = Mesh(np.asarray(devices), ("core",))
    P = PartitionSpec
    nin = n_params + len(out_names)
    sharded = jax.jit(shard_map(body, mesh=mesh, in_specs=(P("core"),) * nin,
                                out_specs=(P("core"),) * len(out_names),
                                check_rep=False))
    concat_in = [np.concatenate([np.asarray(per_core[c][nm]) for c in range(NCORES)], axis=0)
                 for nm in in_names]
    concat_in += [np.concatenate([z] * NCORES, axis=0) for z in zero_outs]
    r = sharded(*concat_in)
    jax.block_until_ready(r)
    best = float("inf")
    for _ in range(batches):
        t0 = _time.perf_counter()
        r = sharded(*concat_in)
        jax.block_until_ready(r)
        dt_s = _time.perf_counter() - t0
        best = min(best, dt_s / reps)
    return best * 1e9
